# revision 6
# baseline (speedup 1.0000x reference)
"""Trainium2 Bass kernel for multi-head causal attention.

Problem: q, k, v of shape [4096, 16, 64] (seq, heads, head_dim) fp32.
  out = softmax(causal(q @ k^T / 8)) @ v, reshaped to [4096, 1024].

Sharding: heads are split across 8 NeuronCores (2 heads per core).
Each core runs the same SPMD Bass program on its own 2 heads; the host
concatenates the per-core [4096, 128] outputs along the feature dim.

Per-core algorithm (flash-attention style, S^T orientation), v2:
  - Stage Q, K as fp16 via SWDGE cast DMA into [128 seq, (h d)] tiles,
    then DMA XBAR-transpose (16x128 tiles, sync queue) into qT/kT
    [128=(h,d), 4096].  The PE does no staging work at all.
  - V per head into vplus [128, 32*65] fp16: 64 V columns plus a ones
    column per 128-row k-block (fused softmax denominator).
  - Main loop over (G, j): one 128-wide k-block j per iteration, both
    heads:
      mm1: S^T[kj, qi] for h0/h1 emitted back-to-back into one combined
           PSUM tile [128, 1024] with tile_position=(h*64, 0) so the two
           K=64 matmuls stream concurrently on disjoint PE row groups.
      exp: split across three engines.  ACT computes exact
           exp(s*0.125) -> fp16.  DVE / GPSIMD compute a Schraudolph
           approximation: t = (s + B/A)*A truncated to int16 and
           bitcast as fp16 equals 2^(s*0.125*log2 e) up to a constant
           factor (cancels in softmax) and a +-2% sawtooth.  For
           diagonal blocks the multiplier A is a precomputed per-element
           tensor (A where causally valid, 0 where masked) so masked
           lanes produce exactly +0.0.  G0 runs on the exact ACT path
           with 0/1 mask multiplies (small-denominator safety).
      mm2: O[qi, 64+1] += expS^T_chunk.T @ vplus_j, deferred two
           iterations (software pipelining keeps the PE queue full so
           the PE p-state can ramp to 2.4 GHz).
  - Normalize: batched reciprocal of the ones-columns (DVE), row-scale
    on GPSIMD, DMA out on the sync queue.
"""

import math

import numpy as np

SEQ = 4096
NHEAD = 16
HDIM = 64
NCORES = 8
HPC = NHEAD // NCORES  # heads per core = 2
SCALE = 0.125

# Schraudolph exp2 constants for fp16 bitcast output.
# t = (s + B/A) * A ; P = bitcast_fp16(int16(t)) ~= C * exp(s * SCALE)
EXP_A = 1024.0 / math.log(2.0) * SCALE  # 184.665
EXP_CORR = -0.0434  # sawtooth centering (constant factor cancels)
EXP_B = 15360.0 + EXP_CORR * 1024.0 + 0.5  # +0.5 centers the truncation
EXP_BOA = EXP_B / EXP_A

_NC_CACHE = {}
LAST_RESULT = {}


def build_attention_nc(seq=SEQ, hpc=HPC, hdim=HDIM, gp_exp=True, split_waits=True):
    """Build the SPMD Bass program for one core handling `hpc` heads."""
    import concourse.bass as bass
    import concourse.mybir as mybir
    import concourse.tile as tile

    f32 = mybir.dt.float32
    fp16 = mybir.dt.float16
    i16 = mybir.dt.int16
    Exp = mybir.ActivationFunctionType.Exp

    assert hpc == 2 and hdim == 64, "layout hardcoded for 2 heads x 64 dim"
    assert seq % 512 == 0
    nt = seq // 128   # number of 128-row seq tiles (32)
    ng = seq // 512   # number of 512-wide q groups (8)

    nc = bass.Bass()
    q = nc.dram_tensor("q", [seq, hpc, hdim], f32, kind="ExternalInput").ap()
    k = nc.dram_tensor("k", [seq, hpc, hdim], f32, kind="ExternalInput").ap()
    v = nc.dram_tensor("v", [seq, hpc, hdim], f32, kind="ExternalOutput" if False else "ExternalInput").ap()
    o = nc.dram_tensor("o", [seq, hpc * hdim], f32, kind="ExternalOutput").ap()

    with tile.TileContext(nc) as tc:
        with (
            tc.tile_pool(name="persist", bufs=1) as persist,
            tc.tile_pool(name="ldstage", bufs=4) as ld_pool,
            tc.tile_pool(name="pexp", bufs=3) as pexp_pool,
            tc.tile_pool(name="outp", bufs=6) as out_pool,
            tc.tile_pool(name="small", bufs=8) as small_pool,
        ):
            # ---- persistent SBUF tensors ----------------------------------
            qT = persist.tile([128, seq], fp16, tag="qT")
            kT = persist.tile([128, seq], fp16, tag="kT")
            vplus = [
                persist.tile([128, nt * (hdim + 1)], fp16, tag=f"vplus{h}", name=f"vplus{h}")
                for h in range(hpc)
            ]
            # amask_t[kj, qi] = EXP_A where kj + 128*t <= qi else 0.0
            # (fused causal mask for the Schraudolph path)
            amask = [persist.tile([128, 512], f32, tag=f"amask{t}", name=f"amask{t}") for t in range(4)]
            # mask01_t: 1/0 causal masks, fp16, for the G0 exact path.
            mask01 = [persist.tile([128, 512], fp16, tag=f"mask01_{t}", name=f"mask01_{t}") for t in range(4)]

            def build_masks():
                for t in range(4):
                    nc.gpsimd.memset(amask[t], EXP_A)
                    nc.gpsimd.affine_select(
                        out=amask[t][:],
                        in_=amask[t][:],
                        compare_op=mybir.AluOpType.is_ge,
                        fill=0.0,
                        base=-128 * t,
                        pattern=[[1, 512]],
                        channel_multiplier=-1,
                    )
                    nc.gpsimd.memset(mask01[t], 1.0)
                    nc.gpsimd.affine_select(
                        out=mask01[t][:],
                        in_=mask01[t][:],
                        compare_op=mybir.AluOpType.is_ge,
                        fill=0.0,
                        base=-128 * t,
                        pattern=[[1, 512]],
                        channel_multiplier=-1,
                    )

            def load_v_chunk(c, tiles_per_chunk):
                # v chunk c covers k-tiles [c*tpc, (c+1)*tpc)
                t0 = c * tiles_per_chunk
                t1 = min(nt, t0 + tiles_per_chunk)
                for h in range(hpc):
                    nc.gpsimd.dma_start(
                        out=vplus[h]
                        .rearrange("p (t x) -> p t x", x=hdim + 1)[:, t0:t1, 0:hdim],
                        in_=v[t0 * 128 : t1 * 128, h, :].rearrange(
                            "(t p) d -> p t d", p=128
                        ),
                    )

            # ---- staging: SWDGE cast-load + DMA XBAR transpose ------------
            chunk = 4  # k-tiles per staged DMA
            nchunks = nt // chunk

            def stage_chunk(src, dstT, c):
                src_r = src.rearrange("(t p) h d -> p t (h d)", p=128)
                st = ld_pool.tile([128, chunk * 128], fp16, tag="ldstage")
                nc.gpsimd.dma_start(
                    out=st.rearrange("p (t x) -> p t x", x=128),
                    in_=src_r[:, c * chunk : (c + 1) * chunk, :],
                )
                for tt in range(chunk):
                    tg = c * chunk + tt
                    nc.sync.dma_start(
                        out=dstT[:, tg * 128 : (tg + 1) * 128],
                        in_=st[:, tt * 128 : (tt + 1) * 128],
                        transpose=True,
                    )

            # memset the ones columns of vplus before any v data lands
            for h in range(hpc):
                nc.vector.memset(vplus[h], 1.0)

            # interleave: k/q chunk pairs (ascending j/G need), v chunks and
            # mask builds woven in so early groups' inputs arrive first.
            for c in range(nchunks):
                stage_chunk(k, kT, c)
                stage_chunk(q, qT, c)
                if c == 0:
                    build_masks()
                # vplus chunk c covers j-tiles c*4..c*4+3, needed by G >= c
                load_v_chunk(c, chunk)

            # ---- main loop -------------------------------------------------
            with (
                tc.tile_pool(name="psum_s", bufs=2, space="PSUM") as psum_s_pool,
                tc.tile_pool(name="psum_o", bufs=2, space="PSUM") as psum_o_pool,
            ):
                _main_loop(
                    nc, mybir, ng, hdim, psum_s_pool, psum_o_pool, pexp_pool,
                    out_pool, small_pool, qT, kT, vplus, amask, mask01, o,
                    hpc, Exp, gp_exp,
                )
    if split_waits:
        _split_multi_waits(nc)
    return nc


def _split_multi_waits(nc):
    """Walrus's codegen accepts at most one sync-wait per instruction on
    this toolchain. Hoist extra waits into standalone single-wait NoOps on
    the same engine queue (same semantics: the sequencer stalls in order)."""
    import concourse.mybir as mybir

    nsplit = 0
    for blk in nc.m.functions[0].blocks:
        newl = []
        for ins in blk.instructions:
            si = getattr(ins, "sync_info", None)
            if si is not None and si.on_wait and len(si.on_wait) > 1:
                waits = list(si.on_wait)
                for w in waits[:-1]:
                    newl.append(
                        mybir.InstNoOp(
                            name=f"{ins.name}-wsplit{nsplit}",
                            sync_info=mybir.SyncInfo(on_wait=[w], on_update=[]),
                            bass_nofuse=True,
                            engine=ins.engine,
                            ins=[],
                            outs=[],
                        )
                    )
                    nsplit += 1
                ins.sync_info = mybir.SyncInfo(
                    on_wait=[waits[-1]], on_update=list(si.on_update or [])
                )
            newl.append(ins)
        blk.instructions = newl
    return nsplit


def _main_loop(nc, mybir, ng, hdim, psum_s_pool, psum_o_pool, pexp_pool,
               out_pool, small_pool, qT, kT, vplus, amask, mask01, o,
               hpc, Exp, gp_exp):
    f32 = mybir.dt.float32
    fp16 = mybir.dt.float16
    i16 = mybir.dt.int16

    add = mybir.AluOpType.add
    mult = mybir.AluOpType.mult

    def emit_mm2(st):
        """Deferred P@V accumulation for one (G, j) iteration."""
        G, j, po, pe, njs, last = st
        t = j - 4 * G
        for h in range(hpc):
            for c in range(4):
                if t > c:
                    continue  # chunk fully masked -> zero contribution
                nc.tensor.matmul(
                    po[h][:, c * 128 : c * 128 + hdim + 1],
                    lhsT=pe[:, h * 512 + c * 128 : h * 512 + (c + 1) * 128],
                    rhs=vplus[h][:, j * 65 : j * 65 + hdim + 1],
                    start=(j == 0 and c == 0),
                    stop=(j == njs - 1 and c == 3),
                    skip_group_check=True,
                )

    Copy = mybir.ActivationFunctionType.Copy

    def emit_finals(G, po):
        # batched reciprocals of the 4 ones-columns per head
        recs = []
        for h in range(hpc):
            rec4 = small_pool.tile([128, 4], f32, tag="rec4", name="rec4")
            nc.vector.reciprocal(
                rec4,
                po[h].rearrange("p (c x) -> p c x", x=128)[:, :, hdim : hdim + 1],
            )
            recs.append(rec4)
        for c in range(4):
            ob = out_pool.tile([128, hpc * hdim], f32, tag="ob", name="ob")
            for h in range(hpc):
                nc.scalar.activation(
                    out=ob[:, h * hdim : (h + 1) * hdim],
                    in_=po[h][:, c * 128 : c * 128 + hdim],
                    func=Copy,
                    scale=recs[h][:, c : c + 1],
                )
            blk = G * 4 + c
            nc.sync.dma_start(out=o[blk * 128 : (blk + 1) * 128, :], in_=ob[:])

    # --- exp emission: greedy ACT/DVE load balancing -------------------
    # ACT: exact exp (PSUM fp32 -> fp16), full jgroup [128, 1024] in one
    # instruction.  DVE: Schraudolph int16-bitcast path; diagonal blocks
    # fuse the causal mask via the amask tensor (masked lanes -> +0.0).
    busy = {"act": 0.0, "dve": 0.0}
    ACT_LAT, DVE_LAT = 217.0, 145.0  # per-instruction latency+seq overhead (ns)

    def exp_act_pair(pe, ps, q0):
        # both heads, one instruction (only valid when q0 == 0)
        nc.scalar.activation(out=pe[:, 0:1024], in_=ps[:, 0:1024], func=Exp, scale=SCALE)
        busy["act"] += 1024 * 0.833 + ACT_LAT

    def exp_dve_pair(pe, ps, t, q0):
        if t >= 0:
            for h in range(2):
                nc.vector.scalar_tensor_tensor(
                    out=pe[:, h * 512 + q0 : (h + 1) * 512].bitcast(i16),
                    in0=ps[:, h * 512 + q0 : (h + 1) * 512],
                    scalar=EXP_BOA,
                    in1=amask[t][:, q0:512],
                    op0=add,
                    op1=mult,
                )
                busy["dve"] += (512 - q0) * 1.042 + DVE_LAT
        else:
            nc.vector.tensor_scalar(
                out=pe[:, 0:1024].bitcast(i16),
                in0=ps[:, 0:1024],
                scalar1=EXP_BOA,
                scalar2=EXP_A,
                op0=add,
                op1=mult,
            )
            busy["dve"] += 1024 * 1.042 + DVE_LAT

    pending = []  # deferred mm2 states (2-deep software pipeline)
    for G in range(ng):
        njs = 4 * G + 4  # causal: k blocks 0 .. 4G+3
        po = [
            psum_o_pool.tile([128, 512], f32, tag=f"po{h}", name=f"po{h}")
            for h in range(hpc)
        ]
        for j in range(njs):
            t = j - 4 * G
            # combined PSUM tile: h0 in cols 0-511, h1 in cols 512-1023
            ps = psum_s_pool.tile([128, 1024], f32, tag="ps", name="ps")
            # Diagonal blocks: columns qi < 128*t are fully masked and only
            # read by skipped mm2 chunks, so mm1 needn't compute them.
            q0 = 128 * t if (t > 0 and G >= 1) else 0
            for h in range(hpc):
                nc.tensor.matmul(
                    ps[:, h * 512 + q0 : (h + 1) * 512],
                    lhsT=kT[h * 64 : (h + 1) * 64, j * 128 : (j + 1) * 128],
                    rhs=qT[h * 64 : (h + 1) * 64, G * 512 + q0 : (G + 1) * 512],
                    start=True,
                    stop=True,
                    tile_position=(h * 64, 0),
                )
            pe = pexp_pool.tile([128, 1024], fp16, tag="pexp", name="pexp")
            if G == 0:
                # exact path with 0/1 mask multiplies (DVE, fp16 2x)
                exp_act_pair(pe, ps, 0)
                for h in range(hpc):
                    nc.vector.tensor_mul(
                        pe[:, h * 512 : (h + 1) * 512],
                        pe[:, h * 512 : (h + 1) * 512],
                        mask01[t][:],
                    )
                    busy["dve"] += 512 * 0.521 + DVE_LAT
            elif t >= 0:
                exp_dve_pair(pe, ps, t, q0)
            elif busy["act"] <= busy["dve"]:
                exp_act_pair(pe, ps, 0)
            else:
                exp_dve_pair(pe, ps, t, 0)
            pending.append((G, j, po, pe, njs, j == njs - 1))
            if len(pending) > 2:
                st = pending.pop(0)
                emit_mm2(st)
                if st[5]:
                    emit_finals(st[0], st[2])
    for st in pending:
        emit_mm2(st)
        if st[5]:
            emit_finals(st[0], st[2])


def _ensure_ntff_hook():
    """The image's antenv package lacks axon_hooks; provide it so
    run_bass_kernel_spmd's trace path works (or degrades gracefully)."""
    import sys
    import types

    try:
        import antenv.axon_hooks  # noqa: F401

        return
    except ImportError:
        pass
    mod = types.ModuleType("antenv.axon_hooks")
    state = {"hook": None}
    mod.set_axon_ntff_profile_hook = lambda h: state.__setitem__("hook", h)
    mod.get_axon_ntff_profile_hook = lambda: state["hook"]
    try:
        from trn_agent_boot.trn_boot import _ntff_profile_via_ctypes

        state["hook"] = _ntff_profile_via_ctypes("/opt/axon/libaxon_pjrt.so")
    except Exception:
        state["hook"] = None
    sys.modules["antenv.axon_hooks"] = mod


def kernel(q, k, v):
    """Full-input entry point: q, k, v [4096, 16, 64] fp32 -> [4096, 1024]."""
    import sys

    if "/opt/trn_rl_repo" not in sys.path:
        sys.path.insert(0, "/opt/trn_rl_repo")
    _ensure_ntff_hook()
    from concourse.bass_utils import run_bass_kernel_spmd

    q = np.asarray(q, dtype=np.float32)
    k = np.asarray(k, dtype=np.float32)
    v = np.asarray(v, dtype=np.float32)
    seq, nhead, hdim = q.shape

    if "nc" not in _NC_CACHE:
        _NC_CACHE["nc"] = build_attention_nc(seq=seq, hpc=HPC, hdim=hdim)
    nc = _NC_CACHE["nc"]

    in_maps = []
    for c in range(NCORES):
        hs = slice(c * HPC, (c + 1) * HPC)
        in_maps.append(
            {
                "q": np.ascontiguousarray(q[:, hs, :]),
                "k": np.ascontiguousarray(k[:, hs, :]),
                "v": np.ascontiguousarray(v[:, hs, :]),
            }
        )
    res = run_bass_kernel_spmd(nc, in_maps, core_ids=list(range(NCORES)))
    LAST_RESULT["exec_time_ns"] = res.exec_time_ns
    try:
        iat = res.instructions_and_trace
        LAST_RESULT["trace_path"] = iat[1] if iat else None
    except Exception:
        LAST_RESULT["trace_path"] = None
    outs = [res.results[c]["o"] for c in range(NCORES)]
    return np.concatenate(outs, axis=1)


# revision 8
# speedup vs baseline: 1.7329x; 1.7329x over previous
"""Trainium2 Bass kernel for multi-head causal attention.

Problem: q, k, v of shape [4096, 16, 64] (seq, heads, head_dim) fp32.
  out = softmax(causal(q @ k^T / 8)) @ v, reshaped to [4096, 1024].

Sharding: heads are split across 8 NeuronCores (2 heads per core).
Each core runs the same SPMD Bass program on its own 2 heads; the host
concatenates the per-core [4096, 128] outputs along the feature dim.

Per-core algorithm (flash-attention style, S^T orientation), v2:
  - Stage Q, K as fp16 via SWDGE cast DMA into [128 seq, (h d)] tiles,
    then DMA XBAR-transpose (16x128 tiles, sync queue) into qT/kT
    [128=(h,d), 4096].  The PE does no staging work at all.
  - V per head into vplus [128, 32*65] fp16: 64 V columns plus a ones
    column per 128-row k-block (fused softmax denominator).
  - Main loop over (G, j): one 128-wide k-block j per iteration, both
    heads:
      mm1: S^T[kj, qi] for h0/h1 emitted back-to-back into one combined
           PSUM tile [128, 1024] with tile_position=(h*64, 0) so the two
           K=64 matmuls stream concurrently on disjoint PE row groups.
      exp: split across three engines.  ACT computes exact
           exp(s*0.125) -> fp16.  DVE / GPSIMD compute a Schraudolph
           approximation: t = (s + B/A)*A truncated to int16 and
           bitcast as fp16 equals 2^(s*0.125*log2 e) up to a constant
           factor (cancels in softmax) and a +-2% sawtooth.  For
           diagonal blocks the multiplier A is a precomputed per-element
           tensor (A where causally valid, 0 where masked) so masked
           lanes produce exactly +0.0.  G0 runs on the exact ACT path
           with 0/1 mask multiplies (small-denominator safety).
      mm2: O[qi, 64+1] += expS^T_chunk.T @ vplus_j, deferred two
           iterations (software pipelining keeps the PE queue full so
           the PE p-state can ramp to 2.4 GHz).
  - Normalize: batched reciprocal of the ones-columns (DVE), row-scale
    on GPSIMD, DMA out on the sync queue.
"""

import math

import numpy as np

SEQ = 4096
NHEAD = 16
HDIM = 64
NCORES = 8
HPC = NHEAD // NCORES  # heads per core = 2
SCALE = 0.125

# Schraudolph exp2 constants for fp16 bitcast output.
# t = (s + B/A) * A ; P = bitcast_fp16(int16(t)) ~= C * exp(s * SCALE)
EXP_A = 1024.0 / math.log(2.0) * SCALE  # 184.665
EXP_CORR = -0.0434  # sawtooth centering (constant factor cancels)
EXP_B = 15360.0 + EXP_CORR * 1024.0 + 0.5  # +0.5 centers the truncation
EXP_BOA = EXP_B / EXP_A

_NC_CACHE = {}
LAST_RESULT = {}


def build_attention_nc(seq=SEQ, hpc=HPC, hdim=HDIM, gp_exp=True, split_waits=True):
    """Build the SPMD Bass program for one core handling `hpc` heads."""
    import concourse.bass as bass
    import concourse.mybir as mybir
    import concourse.tile as tile

    f32 = mybir.dt.float32
    fp16 = mybir.dt.float16
    i16 = mybir.dt.int16
    Exp = mybir.ActivationFunctionType.Exp

    assert hpc == 2 and hdim == 64, "layout hardcoded for 2 heads x 64 dim"
    assert seq % 512 == 0
    nt = seq // 128   # number of 128-row seq tiles (32)
    ng = seq // 512   # number of 512-wide q groups (8)

    nc = bass.Bass()
    q = nc.dram_tensor("q", [seq, hpc, hdim], f32, kind="ExternalInput").ap()
    k = nc.dram_tensor("k", [seq, hpc, hdim], f32, kind="ExternalInput").ap()
    v = nc.dram_tensor("v", [seq, hpc, hdim], f32, kind="ExternalOutput" if False else "ExternalInput").ap()
    o = nc.dram_tensor("o", [seq, hpc * hdim], f32, kind="ExternalOutput").ap()

    with tile.TileContext(nc) as tc:
        with (
            tc.tile_pool(name="persist", bufs=1) as persist,
            tc.tile_pool(name="ldstage", bufs=4) as ld_pool,
            tc.tile_pool(name="pexp", bufs=3) as pexp_pool,
            tc.tile_pool(name="outp", bufs=6) as out_pool,
            tc.tile_pool(name="small", bufs=8) as small_pool,
        ):
            # ---- persistent SBUF tensors ----------------------------------
            qT = persist.tile([128, seq], fp16, tag="qT")
            kT = persist.tile([128, seq], fp16, tag="kT")
            vplus = [
                persist.tile([128, nt * (hdim + 1)], fp16, tag=f"vplus{h}", name=f"vplus{h}")
                for h in range(hpc)
            ]
            # amask_t[kj, qi] = EXP_A where kj + 128*t <= qi else 0.0
            # (fused causal mask for the Schraudolph path)
            amask = [persist.tile([128, 512], f32, tag=f"amask{t}", name=f"amask{t}") for t in range(4)]
            # mask01_t: 1/0 causal masks, fp16, for the G0 exact path.
            mask01 = [persist.tile([128, 512], fp16, tag=f"mask01_{t}", name=f"mask01_{t}") for t in range(4)]

            def build_masks():
                for t in range(4):
                    nc.gpsimd.memset(amask[t], EXP_A)
                    nc.gpsimd.affine_select(
                        out=amask[t][:],
                        in_=amask[t][:],
                        compare_op=mybir.AluOpType.is_ge,
                        fill=0.0,
                        base=-128 * t,
                        pattern=[[1, 512]],
                        channel_multiplier=-1,
                    )
                    nc.gpsimd.memset(mask01[t], 1.0)
                    nc.gpsimd.affine_select(
                        out=mask01[t][:],
                        in_=mask01[t][:],
                        compare_op=mybir.AluOpType.is_ge,
                        fill=0.0,
                        base=-128 * t,
                        pattern=[[1, 512]],
                        channel_multiplier=-1,
                    )

            def load_v_chunk(c, tiles_per_chunk):
                # v chunk c covers k-tiles [c*tpc, (c+1)*tpc)
                t0 = c * tiles_per_chunk
                t1 = min(nt, t0 + tiles_per_chunk)
                for h in range(hpc):
                    nc.gpsimd.dma_start(
                        out=vplus[h]
                        .rearrange("p (t x) -> p t x", x=hdim + 1)[:, t0:t1, 0:hdim],
                        in_=v[t0 * 128 : t1 * 128, h, :].rearrange(
                            "(t p) d -> p t d", p=128
                        ),
                    )

            # ---- staging: SWDGE cast-load + PE transpose ------------------
            # Super-chunks of 8 k-tiles. The PE transposes each staged
            # [128 seq, 128 (h d)] tile into a PSUM buffer borrowed from the
            # mm1 score pool (bitcast fp16), then one wide ACT/DVE copy moves
            # 8 transposed tiles into qT/kT. Staging for super-chunk c+1 is
            # emitted AFTER main-loop groups G=2c,2c+1 so the PE pipeline
            # never serializes behind the whole staging phase.
            schunk = 8
            nsc = nt // schunk  # 4 super-chunks
            identity = persist.tile([128, 128], fp16, tag="identity")
            from concourse.masks import make_identity

            make_identity(nc, identity[:])

            # memset the ones columns of vplus before any v data lands
            for h in range(hpc):
                nc.vector.memset(vplus[h], 1.0)

            with (
                tc.tile_pool(name="psum_s", bufs=2, space="PSUM") as psum_s_pool,
                tc.tile_pool(name="psum_o", bufs=2, space="PSUM") as psum_o_pool,
            ):
                copy_rot = [0]

                def stage_superchunk(c):
                    for src_t, dstT in ((k, kT), (q, qT)):
                        src_r = src_t.rearrange("(t p) h d -> p t (h d)", p=128)
                        st = ld_pool.tile([128, schunk * 128], fp16, tag="ldstage")
                        nc.gpsimd.dma_start(
                            out=st.rearrange("p (t x) -> p t x", x=128),
                            in_=src_r[:, c * schunk : (c + 1) * schunk, :],
                        )
                        tr = psum_s_pool.tile([128, 1024], f32, tag="ps", name="ps")
                        trv = tr.bitcast(fp16)  # [128, 2048] fp16 view
                        for tt in range(schunk):
                            nc.tensor.transpose(
                                trv[:, tt * 128 : (tt + 1) * 128],
                                st[:, tt * 128 : (tt + 1) * 128],
                                identity[:],
                            )
                        # one wide PSUM->SBUF copy per super-chunk, alternating
                        dst = dstT[:, c * 1024 : (c + 1) * 1024]
                        if copy_rot[0] % 2 == 0:
                            nc.scalar.copy(dst, trv[:, 0:1024])
                        else:
                            nc.vector.tensor_copy(dst, trv[:, 0:1024])
                        copy_rot[0] += 1

                def load_v_superchunk(c):
                    t0, t1 = c * schunk, (c + 1) * schunk
                    for h in range(hpc):
                        nc.gpsimd.dma_start(
                            out=vplus[h]
                            .rearrange("p (t x) -> p t x", x=hdim + 1)[:, t0:t1, 0:hdim],
                            in_=v[t0 * 128 : t1 * 128, h, :].rearrange(
                                "(t p) d -> p t d", p=128
                            ),
                        )

                loop = _MainLoop(
                    nc, mybir, ng, hdim, psum_s_pool, psum_o_pool, pexp_pool,
                    out_pool, small_pool, qT, kT, vplus, amask, mask01, o,
                    hpc, Exp,
                )
                for c in range(nsc):
                    stage_superchunk(c)
                    if c == 0:
                        build_masks()
                    load_v_superchunk(c)
                    loop.emit_group(2 * c)
                    loop.emit_group(2 * c + 1)
                loop.flush()
    if split_waits:
        _split_multi_waits(nc)
    return nc


def _split_multi_waits(nc):
    """Walrus's codegen accepts at most one sync-wait per instruction on
    this toolchain. Hoist extra waits into standalone single-wait NoOps on
    the same engine queue (same semantics: the sequencer stalls in order)."""
    import concourse.mybir as mybir

    nsplit = 0
    for blk in nc.m.functions[0].blocks:
        newl = []
        for ins in blk.instructions:
            si = getattr(ins, "sync_info", None)
            if si is not None and si.on_wait and len(si.on_wait) > 1:
                waits = list(si.on_wait)
                for w in waits[:-1]:
                    newl.append(
                        mybir.InstNoOp(
                            name=f"{ins.name}-wsplit{nsplit}",
                            sync_info=mybir.SyncInfo(on_wait=[w], on_update=[]),
                            bass_nofuse=True,
                            engine=ins.engine,
                            ins=[],
                            outs=[],
                        )
                    )
                    nsplit += 1
                ins.sync_info = mybir.SyncInfo(
                    on_wait=[waits[-1]], on_update=list(si.on_update or [])
                )
            newl.append(ins)
        blk.instructions = newl
    return nsplit


class _MainLoop:
    """Emits main-loop groups interleaved with staging.

    One iteration = one 128-wide k-block j for both heads.  mm2 for
    iteration g is deferred until after mm1 of iteration g+2 (the PE
    queue always holds independent work while ACT/DVE compute exp).
    """

    def __init__(self, nc, mybir, ng, hdim, psum_s_pool, psum_o_pool,
                 pexp_pool, out_pool, small_pool, qT, kT, vplus, amask,
                 mask01, o, hpc, Exp):
        self.nc = nc
        self.mybir = mybir
        self.ng = ng
        self.hdim = hdim
        self.psum_s_pool = psum_s_pool
        self.psum_o_pool = psum_o_pool
        self.pexp_pool = pexp_pool
        self.out_pool = out_pool
        self.small_pool = small_pool
        self.qT = qT
        self.kT = kT
        self.vplus = vplus
        self.amask = amask
        self.mask01 = mask01
        self.o = o
        self.hpc = hpc
        self.Exp = Exp
        self.f32 = mybir.dt.float32
        self.fp16 = mybir.dt.float16
        self.i16 = mybir.dt.int16
        self.add = mybir.AluOpType.add
        self.mult = mybir.AluOpType.mult
        self.Copy = mybir.ActivationFunctionType.Copy
        self.pending = []
        # greedy ACT/DVE balance counters (estimated busy ns)
        self.busy = {"act": 0.0, "dve": 0.0}
        self.ACT_LAT = 217.0
        self.DVE_LAT = 145.0

    # --- exp paths -----------------------------------------------------
    def exp_act_pair(self, pe, ps):
        nc = self.nc
        nc.scalar.activation(out=pe[:, 0:1024], in_=ps[:, 0:1024],
                             func=self.Exp, scale=SCALE)
        self.busy["act"] += 1024 * 0.833 + self.ACT_LAT

    def exp_dve_pair(self, pe, ps, t, q0):
        nc = self.nc
        if t >= 0:
            for h in range(2):
                nc.vector.scalar_tensor_tensor(
                    out=pe[:, h * 512 + q0 : (h + 1) * 512].bitcast(self.i16),
                    in0=ps[:, h * 512 + q0 : (h + 1) * 512],
                    scalar=EXP_BOA,
                    in1=self.amask[t][:, q0:512],
                    op0=self.add,
                    op1=self.mult,
                )
                self.busy["dve"] += (512 - q0) * 1.042 + self.DVE_LAT
        else:
            nc.vector.tensor_scalar(
                out=pe[:, 0:1024].bitcast(self.i16),
                in0=ps[:, 0:1024],
                scalar1=EXP_BOA,
                scalar2=EXP_A,
                op0=self.add,
                op1=self.mult,
            )
            self.busy["dve"] += 1024 * 1.042 + self.DVE_LAT

    # --- mm2 + finals --------------------------------------------------
    def emit_mm2(self, st):
        nc = self.nc
        G, j, po, pe, njs, last = st
        t = j - 4 * G
        hdim = self.hdim
        for h in range(self.hpc):
            for c in range(4):
                if t > c:
                    continue  # chunk fully masked -> zero contribution
                nc.tensor.matmul(
                    po[h][:, c * 128 : c * 128 + hdim + 1],
                    lhsT=pe[:, h * 512 + c * 128 : h * 512 + (c + 1) * 128],
                    rhs=self.vplus[h][:, j * 65 : j * 65 + hdim + 1],
                    start=(j == 0 and c == 0),
                    stop=(j == njs - 1 and c == 3),
                    skip_group_check=True,
                )

    def emit_finals(self, G, po):
        nc = self.nc
        hdim = self.hdim
        recs = []
        for h in range(self.hpc):
            rec4 = self.small_pool.tile([128, 4], self.f32, tag="rec4", name="rec4")
            nc.vector.reciprocal(
                rec4,
                po[h].rearrange("p (c x) -> p c x", x=128)[:, :, hdim : hdim + 1],
            )
            recs.append(rec4)
        for c in range(4):
            ob = self.out_pool.tile([128, self.hpc * hdim], self.f32, tag="ob", name="ob")
            for h in range(self.hpc):
                nc.scalar.activation(
                    out=ob[:, h * hdim : (h + 1) * hdim],
                    in_=po[h][:, c * 128 : c * 128 + hdim],
                    func=self.Copy,
                    scale=recs[h][:, c : c + 1],
                )
                self.busy["act"] += 64 * 0.833 + self.ACT_LAT
            blk = G * 4 + c
            nc.sync.dma_start(out=self.o[blk * 128 : (blk + 1) * 128, :], in_=ob[:])

    # --- per-group emission --------------------------------------------
    def emit_group(self, G):
        nc = self.nc
        njs = 4 * G + 4
        po = [
            self.psum_o_pool.tile([128, 512], self.f32, tag=f"po{h}", name=f"po{h}")
            for h in range(self.hpc)
        ]
        for j in range(njs):
            t = j - 4 * G
            ps = self.psum_s_pool.tile([128, 1024], self.f32, tag="ps", name="ps")
            q0 = 128 * t if (t > 0 and G >= 1) else 0
            for h in range(self.hpc):
                nc.tensor.matmul(
                    ps[:, h * 512 + q0 : (h + 1) * 512],
                    lhsT=self.kT[h * 64 : (h + 1) * 64, j * 128 : (j + 1) * 128],
                    rhs=self.qT[h * 64 : (h + 1) * 64, G * 512 + q0 : (G + 1) * 512],
                    start=True,
                    stop=True,
                    tile_position=(h * 64, 0),
                )
            pe = self.pexp_pool.tile([128, 1024], self.fp16, tag="pexp", name="pexp")
            if G == 0:
                # exact path with 0/1 mask multiplies (DVE, fp16 2x)
                self.exp_act_pair(pe, ps)
                for h in range(self.hpc):
                    nc.vector.tensor_mul(
                        pe[:, h * 512 : (h + 1) * 512],
                        pe[:, h * 512 : (h + 1) * 512],
                        self.mask01[t][:],
                    )
                    self.busy["dve"] += 512 * 0.521 + self.DVE_LAT
            elif t >= 0:
                self.exp_dve_pair(pe, ps, t, q0)
            elif self.busy["act"] <= self.busy["dve"]:
                self.exp_act_pair(pe, ps)
            else:
                self.exp_dve_pair(pe, ps, t, 0)
            self.pending.append((G, j, po, pe, njs, j == njs - 1))
            if len(self.pending) > 2:
                st = self.pending.pop(0)
                self.emit_mm2(st)
                if st[5]:
                    self.emit_finals(st[0], st[2])

    def flush(self):
        for st in self.pending:
            self.emit_mm2(st)
            if st[5]:
                self.emit_finals(st[0], st[2])
        self.pending = []


def _ensure_ntff_hook():
    """The image's antenv package lacks axon_hooks; provide it so
    run_bass_kernel_spmd's trace path works (or degrades gracefully)."""
    import sys
    import types

    try:
        import antenv.axon_hooks  # noqa: F401

        return
    except ImportError:
        pass
    mod = types.ModuleType("antenv.axon_hooks")
    state = {"hook": None}
    mod.set_axon_ntff_profile_hook = lambda h: state.__setitem__("hook", h)
    mod.get_axon_ntff_profile_hook = lambda: state["hook"]
    try:
        from trn_agent_boot.trn_boot import _ntff_profile_via_ctypes

        state["hook"] = _ntff_profile_via_ctypes("/opt/axon/libaxon_pjrt.so")
    except Exception:
        state["hook"] = None
    sys.modules["antenv.axon_hooks"] = mod


def kernel(q, k, v):
    """Full-input entry point: q, k, v [4096, 16, 64] fp32 -> [4096, 1024]."""
    import sys

    if "/opt/trn_rl_repo" not in sys.path:
        sys.path.insert(0, "/opt/trn_rl_repo")
    _ensure_ntff_hook()
    from concourse.bass_utils import run_bass_kernel_spmd

    q = np.asarray(q, dtype=np.float32)
    k = np.asarray(k, dtype=np.float32)
    v = np.asarray(v, dtype=np.float32)
    seq, nhead, hdim = q.shape

    if "nc" not in _NC_CACHE:
        _NC_CACHE["nc"] = build_attention_nc(seq=seq, hpc=HPC, hdim=hdim)
    nc = _NC_CACHE["nc"]

    in_maps = []
    for c in range(NCORES):
        hs = slice(c * HPC, (c + 1) * HPC)
        in_maps.append(
            {
                "q": np.ascontiguousarray(q[:, hs, :]),
                "k": np.ascontiguousarray(k[:, hs, :]),
                "v": np.ascontiguousarray(v[:, hs, :]),
            }
        )
    res = run_bass_kernel_spmd(nc, in_maps, core_ids=list(range(NCORES)))
    LAST_RESULT["exec_time_ns"] = res.exec_time_ns
    try:
        iat = res.instructions_and_trace
        LAST_RESULT["trace_path"] = iat[1] if iat else None
    except Exception:
        LAST_RESULT["trace_path"] = None
    outs = [res.results[c]["o"] for c in range(NCORES)]
    return np.concatenate(outs, axis=1)


# revision 10
# speedup vs baseline: 1.9100x; 1.1022x over previous
"""Trainium2 Bass kernel for multi-head causal attention.

Problem: q, k, v of shape [4096, 16, 64] (seq, heads, head_dim) fp32.
  out = softmax(causal(q @ k^T / 8)) @ v, reshaped to [4096, 1024].

Sharding: heads are split across 8 NeuronCores (2 heads per core).
Each core runs the same SPMD Bass program on its own 2 heads; the host
concatenates the per-core [4096, 128] outputs along the feature dim.

Per-core algorithm (flash-attention style, S^T orientation), v2:
  - Stage Q, K as fp16 via SWDGE cast DMA into [128 seq, (h d)] tiles,
    then DMA XBAR-transpose (16x128 tiles, sync queue) into qT/kT
    [128=(h,d), 4096].  The PE does no staging work at all.
  - V per head into vplus [128, 32*65] fp16: 64 V columns plus a ones
    column per 128-row k-block (fused softmax denominator).
  - Main loop over (G, j): one 128-wide k-block j per iteration, both
    heads:
      mm1: S^T[kj, qi] for h0/h1 emitted back-to-back into one combined
           PSUM tile [128, 1024] with tile_position=(h*64, 0) so the two
           K=64 matmuls stream concurrently on disjoint PE row groups.
      exp: split across three engines.  ACT computes exact
           exp(s*0.125) -> fp16.  DVE / GPSIMD compute a Schraudolph
           approximation: t = (s + B/A)*A truncated to int16 and
           bitcast as fp16 equals 2^(s*0.125*log2 e) up to a constant
           factor (cancels in softmax) and a +-2% sawtooth.  For
           diagonal blocks the multiplier A is a precomputed per-element
           tensor (A where causally valid, 0 where masked) so masked
           lanes produce exactly +0.0.  G0 runs on the exact ACT path
           with 0/1 mask multiplies (small-denominator safety).
      mm2: O[qi, 64+1] += expS^T_chunk.T @ vplus_j, deferred two
           iterations (software pipelining keeps the PE queue full so
           the PE p-state can ramp to 2.4 GHz).
  - Normalize: batched reciprocal of the ones-columns (DVE), row-scale
    on GPSIMD, DMA out on the sync queue.
"""

import math

import numpy as np

SEQ = 4096
NHEAD = 16
HDIM = 64
NCORES = 8
HPC = NHEAD // NCORES  # heads per core = 2
SCALE = 0.125

# Schraudolph exp2 constants for fp16 bitcast output.
# t = (s + B/A) * A ; P = bitcast_fp16(int16(t)) ~= C * exp(s * SCALE)
EXP_A = 1024.0 / math.log(2.0) * SCALE  # 184.665
EXP_CORR = -0.0434  # sawtooth centering (constant factor cancels)
EXP_B = 15360.0 + EXP_CORR * 1024.0 + 0.5  # +0.5 centers the truncation
EXP_BOA = EXP_B / EXP_A

_NC_CACHE = {}
LAST_RESULT = {}


def build_attention_nc(seq=SEQ, hpc=HPC, hdim=HDIM, gp_exp=True, split_waits=True):
    """Build the SPMD Bass program for one core handling `hpc` heads."""
    import concourse.bass as bass
    import concourse.mybir as mybir
    import concourse.tile as tile

    f32 = mybir.dt.float32
    fp16 = mybir.dt.float16
    i16 = mybir.dt.int16
    Exp = mybir.ActivationFunctionType.Exp

    assert hpc == 2 and hdim == 64, "layout hardcoded for 2 heads x 64 dim"
    assert seq % 512 == 0
    nt = seq // 128   # number of 128-row seq tiles (32)
    ng = seq // 512   # number of 512-wide q groups (8)

    nc = bass.Bass()
    q = nc.dram_tensor("q", [seq, hpc, hdim], f32, kind="ExternalInput").ap()
    k = nc.dram_tensor("k", [seq, hpc, hdim], f32, kind="ExternalInput").ap()
    v = nc.dram_tensor("v", [seq, hpc, hdim], f32, kind="ExternalOutput" if False else "ExternalInput").ap()
    o = nc.dram_tensor("o", [seq, hpc * hdim], f32, kind="ExternalOutput").ap()

    with tile.TileContext(nc) as tc:
        with (
            tc.tile_pool(name="persist", bufs=1) as persist,
            tc.tile_pool(name="ldstage", bufs=4) as ld_pool,
            tc.tile_pool(name="pexp", bufs=3) as pexp_pool,
            tc.tile_pool(name="outp", bufs=6) as out_pool,
            tc.tile_pool(name="small", bufs=8) as small_pool,
        ):
            # ---- persistent SBUF tensors ----------------------------------
            qT = persist.tile([128, seq], fp16, tag="qT")
            kT = persist.tile([128, seq], fp16, tag="kT")
            vplus = [
                persist.tile([128, nt * (hdim + 1)], fp16, tag=f"vplus{h}", name=f"vplus{h}")
                for h in range(hpc)
            ]
            # amask_t[kj, qi] = EXP_A where kj + 128*t <= qi else 0.0
            # (fused causal mask for the Schraudolph path)
            amask = [persist.tile([128, 512], f32, tag=f"amask{t}", name=f"amask{t}") for t in range(4)]
            # mask01_t: 1/0 causal masks, fp16, for the G0 exact path.
            mask01 = [persist.tile([128, 512], fp16, tag=f"mask01_{t}", name=f"mask01_{t}") for t in range(4)]

            def build_masks():
                for t in range(4):
                    nc.gpsimd.memset(amask[t], EXP_A)
                    nc.gpsimd.affine_select(
                        out=amask[t][:],
                        in_=amask[t][:],
                        compare_op=mybir.AluOpType.is_ge,
                        fill=0.0,
                        base=-128 * t,
                        pattern=[[1, 512]],
                        channel_multiplier=-1,
                    )
                    nc.gpsimd.memset(mask01[t], 1.0)
                    nc.gpsimd.affine_select(
                        out=mask01[t][:],
                        in_=mask01[t][:],
                        compare_op=mybir.AluOpType.is_ge,
                        fill=0.0,
                        base=-128 * t,
                        pattern=[[1, 512]],
                        channel_multiplier=-1,
                    )

            def load_v_chunk(c, tiles_per_chunk):
                # v chunk c covers k-tiles [c*tpc, (c+1)*tpc)
                t0 = c * tiles_per_chunk
                t1 = min(nt, t0 + tiles_per_chunk)
                for h in range(hpc):
                    nc.gpsimd.dma_start(
                        out=vplus[h]
                        .rearrange("p (t x) -> p t x", x=hdim + 1)[:, t0:t1, 0:hdim],
                        in_=v[t0 * 128 : t1 * 128, h, :].rearrange(
                            "(t p) d -> p t d", p=128
                        ),
                    )

            # ---- staging: SWDGE cast-load + PE transpose ------------------
            # Super-chunks of 8 k-tiles. The PE transposes each staged
            # [128 seq, 128 (h d)] tile into a PSUM buffer borrowed from the
            # mm1 score pool (bitcast fp16), then one wide ACT/DVE copy moves
            # 8 transposed tiles into qT/kT. Staging for super-chunk c+1 is
            # emitted AFTER main-loop groups G=2c,2c+1 so the PE pipeline
            # never serializes behind the whole staging phase.
            schunk = 8
            nsc = nt // schunk  # 4 super-chunks
            identity = persist.tile([128, 128], fp16, tag="identity")
            from concourse.masks import make_identity

            make_identity(nc, identity[:])

            # memset the ones columns of vplus before any v data lands
            for h in range(hpc):
                nc.vector.memset(vplus[h], 1.0)

            with (
                tc.tile_pool(name="psum_s", bufs=3, space="PSUM") as psum_s_pool,
                tc.tile_pool(name="psum_o", bufs=1, space="PSUM") as psum_o_pool,
            ):
                copy_rot = [0]

                def stage_superchunk(c):
                    for src_t, dstT in ((k, kT), (q, qT)):
                        src_r = src_t.rearrange("(t p) h d -> p t (h d)", p=128)
                        st = ld_pool.tile([128, schunk * 128], fp16, tag="ldstage")
                        nc.gpsimd.dma_start(
                            out=st.rearrange("p (t x) -> p t x", x=128),
                            in_=src_r[:, c * schunk : (c + 1) * schunk, :],
                        )
                        tr = psum_s_pool.tile([128, 1024], f32, tag="ps", name="ps")
                        trv = tr.bitcast(fp16)  # [128, 2048] fp16 view
                        for tt in range(schunk):
                            nc.tensor.transpose(
                                trv[:, tt * 128 : (tt + 1) * 128],
                                st[:, tt * 128 : (tt + 1) * 128],
                                identity[:],
                            )
                        # one wide PSUM->SBUF copy per super-chunk, alternating
                        dst = dstT[:, c * 1024 : (c + 1) * 1024]
                        if copy_rot[0] % 2 == 0:
                            nc.scalar.copy(dst, trv[:, 0:1024])
                        else:
                            nc.vector.tensor_copy(dst, trv[:, 0:1024])
                        copy_rot[0] += 1

                def load_v_superchunk(c):
                    t0, t1 = c * schunk, (c + 1) * schunk
                    for h in range(hpc):
                        nc.gpsimd.dma_start(
                            out=vplus[h]
                            .rearrange("p (t x) -> p t x", x=hdim + 1)[:, t0:t1, 0:hdim],
                            in_=v[t0 * 128 : t1 * 128, h, :].rearrange(
                                "(t p) d -> p t d", p=128
                            ),
                        )

                loop = _MainLoop(
                    nc, mybir, ng, hdim, psum_s_pool, psum_o_pool, pexp_pool,
                    out_pool, small_pool, qT, kT, vplus, amask, mask01, o,
                    hpc, Exp,
                )
                for c in range(nsc):
                    stage_superchunk(c)
                    if c == 0:
                        build_masks()
                    load_v_superchunk(c)
                    loop.emit_group(2 * c)
                    loop.emit_group(2 * c + 1)
                loop.flush()
    if split_waits:
        _split_multi_waits(nc)
    return nc


def _split_multi_waits(nc):
    """Walrus's codegen accepts at most one sync-wait per instruction on
    this toolchain. Hoist extra waits into standalone single-wait NoOps on
    the same engine queue (same semantics: the sequencer stalls in order)."""
    import concourse.mybir as mybir

    nsplit = 0
    for blk in nc.m.functions[0].blocks:
        newl = []
        for ins in blk.instructions:
            si = getattr(ins, "sync_info", None)
            if si is not None and si.on_wait and len(si.on_wait) > 1:
                waits = list(si.on_wait)
                for w in waits[:-1]:
                    newl.append(
                        mybir.InstNoOp(
                            name=f"{ins.name}-wsplit{nsplit}",
                            sync_info=mybir.SyncInfo(on_wait=[w], on_update=[]),
                            bass_nofuse=True,
                            engine=ins.engine,
                            ins=[],
                            outs=[],
                        )
                    )
                    nsplit += 1
                ins.sync_info = mybir.SyncInfo(
                    on_wait=[waits[-1]], on_update=list(si.on_update or [])
                )
            newl.append(ins)
        blk.instructions = newl
    return nsplit


class _MainLoop:
    """Emits main-loop groups interleaved with staging.

    One iteration = one 128-wide k-block j for both heads.  mm2 for
    iteration g is deferred until after mm1 of iteration g+2 (the PE
    queue always holds independent work while ACT/DVE compute exp).
    """

    def __init__(self, nc, mybir, ng, hdim, psum_s_pool, psum_o_pool,
                 pexp_pool, out_pool, small_pool, qT, kT, vplus, amask,
                 mask01, o, hpc, Exp):
        self.nc = nc
        self.mybir = mybir
        self.ng = ng
        self.hdim = hdim
        self.psum_s_pool = psum_s_pool
        self.psum_o_pool = psum_o_pool
        self.pexp_pool = pexp_pool
        self.out_pool = out_pool
        self.small_pool = small_pool
        self.qT = qT
        self.kT = kT
        self.vplus = vplus
        self.amask = amask
        self.mask01 = mask01
        self.o = o
        self.hpc = hpc
        self.Exp = Exp
        self.f32 = mybir.dt.float32
        self.fp16 = mybir.dt.float16
        self.i16 = mybir.dt.int16
        self.add = mybir.AluOpType.add
        self.mult = mybir.AluOpType.mult
        self.Copy = mybir.ActivationFunctionType.Copy
        self.pending = []
        # greedy ACT/DVE balance counters (estimated busy ns)
        self.busy = {"act": 0.0, "dve": 0.0}
        self.ACT_LAT = 420.0
        self.DVE_LAT = 145.0

    # --- exp paths -----------------------------------------------------
    def exp_act_pair(self, pe, ps):
        nc = self.nc
        nc.scalar.activation(out=pe[:, 0:1024], in_=ps[:, 0:1024],
                             func=self.Exp, scale=SCALE)
        self.busy["act"] += 1024 * 0.833 + self.ACT_LAT

    def exp_dve_pair(self, pe, ps, t, q0):
        nc = self.nc
        if t >= 0:
            for h in range(2):
                nc.vector.scalar_tensor_tensor(
                    out=pe[:, h * 512 + q0 : (h + 1) * 512].bitcast(self.i16),
                    in0=ps[:, h * 512 + q0 : (h + 1) * 512],
                    scalar=EXP_BOA,
                    in1=self.amask[t][:, q0:512],
                    op0=self.add,
                    op1=self.mult,
                )
                self.busy["dve"] += (512 - q0) * 1.042 + self.DVE_LAT
        else:
            nc.vector.tensor_scalar(
                out=pe[:, 0:1024].bitcast(self.i16),
                in0=ps[:, 0:1024],
                scalar1=EXP_BOA,
                scalar2=EXP_A,
                op0=self.add,
                op1=self.mult,
            )
            self.busy["dve"] += 1024 * 1.042 + self.DVE_LAT

    # --- mm2 + finals --------------------------------------------------
    def emit_mm2(self, st):
        nc = self.nc
        G, j, po, pe, njs, last = st
        t = j - 4 * G
        hdim = self.hdim
        for h in range(self.hpc):
            for c in range(4):
                if t > c:
                    continue  # chunk fully masked -> zero contribution
                nc.tensor.matmul(
                    po[h][:, c * 128 : c * 128 + hdim + 1],
                    lhsT=pe[:, h * 512 + c * 128 : h * 512 + (c + 1) * 128],
                    rhs=self.vplus[h][:, j * 65 : j * 65 + hdim + 1],
                    start=(j == 0 and c == 0),
                    stop=(j == njs - 1 and c == 3),
                    skip_group_check=True,
                )

    def emit_finals(self, G, po):
        # Copy po out of PSUM immediately (frees the bank for the next G's
        # mm2 accumulation), then do reciprocal+normalize from SBUF so the
        # normalize can run on the otherwise-idle GPSIMD engine.
        nc = self.nc
        hdim = self.hdim
        pos = []
        for h in range(self.hpc):
            p_sb = self.out_pool.tile([128, 260], self.f32, tag="posb", name="posb")
            src_ap = po[h].rearrange("p (c x) -> p c x", x=128)[:, :, 0 : hdim + 1]
            dst_ap = p_sb.rearrange("p (c x) -> p c x", x=hdim + 1)
            if self.busy["act"] <= self.busy["dve"]:
                nc.scalar.copy(dst_ap, src_ap)
                self.busy["act"] += 260 * 0.833 + self.ACT_LAT
            else:
                nc.vector.tensor_copy(dst_ap, src_ap)
                self.busy["dve"] += 260 * 1.042 + self.DVE_LAT
            pos.append(p_sb)
        recs = []
        for h in range(self.hpc):
            rec4 = self.small_pool.tile([128, 4], self.f32, tag="rec4", name="rec4")
            nc.vector.reciprocal(
                rec4,
                pos[h].rearrange("p (c x) -> p c x", x=hdim + 1)[:, :, hdim : hdim + 1],
            )
            recs.append(rec4)
        for c in range(4):
            ob = self.out_pool.tile([128, self.hpc * hdim], self.f32, tag="ob", name="ob")
            for h in range(self.hpc):
                nc.gpsimd.tensor_scalar(
                    out=ob[:, h * hdim : (h + 1) * hdim],
                    in0=pos[h][:, c * (hdim + 1) : c * (hdim + 1) + hdim],
                    scalar1=recs[h][:, c : c + 1],
                    scalar2=None,
                    op0=self.mult,
                )
            blk = G * 4 + c
            nc.sync.dma_start(out=self.o[blk * 128 : (blk + 1) * 128, :], in_=ob[:])

    # --- per-group emission --------------------------------------------
    def emit_group(self, G):
        nc = self.nc
        njs = 4 * G + 4
        po = [
            self.psum_o_pool.tile([128, 512], self.f32, tag=f"po{h}", name=f"po{h}")
            for h in range(self.hpc)
        ]
        for j in range(njs):
            t = j - 4 * G
            ps = self.psum_s_pool.tile([128, 1024], self.f32, tag="ps", name="ps")
            q0 = 128 * t if (t > 0 and G >= 1) else 0
            for h in range(self.hpc):
                nc.tensor.matmul(
                    ps[:, h * 512 + q0 : (h + 1) * 512],
                    lhsT=self.kT[h * 64 : (h + 1) * 64, j * 128 : (j + 1) * 128],
                    rhs=self.qT[h * 64 : (h + 1) * 64, G * 512 + q0 : (G + 1) * 512],
                    start=True,
                    stop=True,
                    tile_position=(h * 64, 0),
                )
            pe = self.pexp_pool.tile([128, 1024], self.fp16, tag="pexp", name="pexp")
            if G == 0:
                # exact path with 0/1 mask multiplies (DVE, fp16 2x)
                self.exp_act_pair(pe, ps)
                for h in range(self.hpc):
                    nc.gpsimd.tensor_mul(
                        pe[:, h * 512 : (h + 1) * 512],
                        pe[:, h * 512 : (h + 1) * 512],
                        self.mask01[t][:],
                    )
            elif t >= 0:
                self.exp_dve_pair(pe, ps, t, q0)
            elif self.busy["act"] <= self.busy["dve"]:
                self.exp_act_pair(pe, ps)
            else:
                self.exp_dve_pair(pe, ps, t, 0)
            self.pending.append((G, j, po, pe, njs, j == njs - 1))
            if len(self.pending) > 2:
                st = self.pending.pop(0)
                self.emit_mm2(st)
                if st[5]:
                    self.emit_finals(st[0], st[2])

    def flush(self):
        for st in self.pending:
            self.emit_mm2(st)
            if st[5]:
                self.emit_finals(st[0], st[2])
        self.pending = []


def _ensure_ntff_hook():
    """The image's antenv package lacks axon_hooks; provide it so
    run_bass_kernel_spmd's trace path works (or degrades gracefully)."""
    import sys
    import types

    try:
        import antenv.axon_hooks  # noqa: F401

        return
    except ImportError:
        pass
    mod = types.ModuleType("antenv.axon_hooks")
    state = {"hook": None}
    mod.set_axon_ntff_profile_hook = lambda h: state.__setitem__("hook", h)
    mod.get_axon_ntff_profile_hook = lambda: state["hook"]
    try:
        from trn_agent_boot.trn_boot import _ntff_profile_via_ctypes

        state["hook"] = _ntff_profile_via_ctypes("/opt/axon/libaxon_pjrt.so")
    except Exception:
        state["hook"] = None
    sys.modules["antenv.axon_hooks"] = mod


def kernel(q, k, v):
    """Full-input entry point: q, k, v [4096, 16, 64] fp32 -> [4096, 1024]."""
    import sys

    if "/opt/trn_rl_repo" not in sys.path:
        sys.path.insert(0, "/opt/trn_rl_repo")
    _ensure_ntff_hook()
    from concourse.bass_utils import run_bass_kernel_spmd

    q = np.asarray(q, dtype=np.float32)
    k = np.asarray(k, dtype=np.float32)
    v = np.asarray(v, dtype=np.float32)
    seq, nhead, hdim = q.shape

    if "nc" not in _NC_CACHE:
        _NC_CACHE["nc"] = build_attention_nc(seq=seq, hpc=HPC, hdim=hdim)
    nc = _NC_CACHE["nc"]

    in_maps = []
    for c in range(NCORES):
        hs = slice(c * HPC, (c + 1) * HPC)
        in_maps.append(
            {
                "q": np.ascontiguousarray(q[:, hs, :]),
                "k": np.ascontiguousarray(k[:, hs, :]),
                "v": np.ascontiguousarray(v[:, hs, :]),
            }
        )
    res = run_bass_kernel_spmd(nc, in_maps, core_ids=list(range(NCORES)))
    LAST_RESULT["exec_time_ns"] = res.exec_time_ns
    try:
        iat = res.instructions_and_trace
        LAST_RESULT["trace_path"] = iat[1] if iat else None
    except Exception:
        LAST_RESULT["trace_path"] = None
    outs = [res.results[c]["o"] for c in range(NCORES)]
    return np.concatenate(outs, axis=1)


# revision 14
# speedup vs baseline: 1.9775x; 1.0353x over previous
"""Trainium2 Bass kernel for multi-head causal attention.

Problem: q, k, v of shape [4096, 16, 64] (seq, heads, head_dim) fp32.
  out = softmax(causal(q @ k^T / 8)) @ v, reshaped to [4096, 1024].

Sharding: heads are split across 8 NeuronCores (2 heads per core).
Each core runs the same SPMD Bass program on its own 2 heads; the host
concatenates the per-core [4096, 128] outputs along the feature dim.

Per-core algorithm (flash-attention style, S^T orientation), v2:
  - Stage Q, K as fp16 via SWDGE cast DMA into [128 seq, (h d)] tiles,
    then DMA XBAR-transpose (16x128 tiles, sync queue) into qT/kT
    [128=(h,d), 4096].  The PE does no staging work at all.
  - V per head into vplus [128, 32*65] fp16: 64 V columns plus a ones
    column per 128-row k-block (fused softmax denominator).
  - Main loop over (G, j): one 128-wide k-block j per iteration, both
    heads:
      mm1: S^T[kj, qi] for h0/h1 emitted back-to-back into one combined
           PSUM tile [128, 1024] with tile_position=(h*64, 0) so the two
           K=64 matmuls stream concurrently on disjoint PE row groups.
      exp: split across three engines.  ACT computes exact
           exp(s*0.125) -> fp16.  DVE / GPSIMD compute a Schraudolph
           approximation: t = (s + B/A)*A truncated to int16 and
           bitcast as fp16 equals 2^(s*0.125*log2 e) up to a constant
           factor (cancels in softmax) and a +-2% sawtooth.  For
           diagonal blocks the multiplier A is a precomputed per-element
           tensor (A where causally valid, 0 where masked) so masked
           lanes produce exactly +0.0.  G0 runs on the exact ACT path
           with 0/1 mask multiplies (small-denominator safety).
      mm2: O[qi, 64+1] += expS^T_chunk.T @ vplus_j, deferred two
           iterations (software pipelining keeps the PE queue full so
           the PE p-state can ramp to 2.4 GHz).
  - Normalize: batched reciprocal of the ones-columns (DVE), row-scale
    on GPSIMD, DMA out on the sync queue.
"""

import math

import numpy as np

SEQ = 4096
NHEAD = 16
HDIM = 64
NCORES = 8
HPC = NHEAD // NCORES  # heads per core = 2
SCALE = 0.125

# Schraudolph exp2 constants for fp16 bitcast output.
# t = (s + B/A) * A ; P = bitcast_fp16(int16(t)) ~= C * exp(s * SCALE)
EXP_A = 1024.0 / math.log(2.0) * SCALE  # 184.665
EXP_CORR = -0.0434  # sawtooth centering (constant factor cancels)
EXP_B = 15360.0 + EXP_CORR * 1024.0 + 0.5  # +0.5 centers the truncation
EXP_BOA = EXP_B / EXP_A

_NC_CACHE = {}
LAST_RESULT = {}


def build_attention_nc(seq=SEQ, hpc=HPC, hdim=HDIM, gp_exp=True, split_waits=True):
    """Build the SPMD Bass program for one core handling `hpc` heads."""
    import concourse.bass as bass
    import concourse.mybir as mybir
    import concourse.tile as tile

    f32 = mybir.dt.float32
    fp16 = mybir.dt.float16
    i16 = mybir.dt.int16
    Exp = mybir.ActivationFunctionType.Exp

    assert hpc == 2 and hdim == 64, "layout hardcoded for 2 heads x 64 dim"
    assert seq % 512 == 0
    nt = seq // 128   # number of 128-row seq tiles (32)
    ng = seq // 512   # number of 512-wide q groups (8)

    nc = bass.Bass()
    q = nc.dram_tensor("q", [seq, hpc, hdim], f32, kind="ExternalInput").ap()
    k = nc.dram_tensor("k", [seq, hpc, hdim], f32, kind="ExternalInput").ap()
    v = nc.dram_tensor("v", [seq, hpc, hdim], f32, kind="ExternalOutput" if False else "ExternalInput").ap()
    o = nc.dram_tensor("o", [seq, hpc * hdim], f32, kind="ExternalOutput").ap()

    with tile.TileContext(nc) as tc:
        with (
            tc.tile_pool(name="persist", bufs=1) as persist,
            tc.tile_pool(name="ldstage", bufs=4) as ld_pool,
            tc.tile_pool(name="pexp", bufs=3) as pexp_pool,
            tc.tile_pool(name="outp", bufs=6) as out_pool,
            tc.tile_pool(name="small", bufs=8) as small_pool,
        ):
            # ---- persistent SBUF tensors ----------------------------------
            qT = persist.tile([128, seq], fp16, tag="qT")
            kT = persist.tile([128, seq], fp16, tag="kT")
            vplus = [
                persist.tile([128, nt * (hdim + 1)], fp16, tag=f"vplus{h}", name=f"vplus{h}")
                for h in range(hpc)
            ]
            # amask_t[kj, qi] = EXP_A where kj + 128*t <= qi else 0.0
            # (fused causal mask for the Schraudolph path)
            amask = [persist.tile([128, 512], f32, tag=f"amask{t}", name=f"amask{t}") for t in range(4)]
            # mask01_t: 1/0 causal masks, fp16, for the G0 exact path.
            mask01 = [persist.tile([128, 512], fp16, tag=f"mask01_{t}", name=f"mask01_{t}") for t in range(4)]

            def build_masks():
                for t in range(4):
                    nc.gpsimd.memset(amask[t], EXP_A)
                    nc.gpsimd.affine_select(
                        out=amask[t][:],
                        in_=amask[t][:],
                        compare_op=mybir.AluOpType.is_ge,
                        fill=0.0,
                        base=-128 * t,
                        pattern=[[1, 512]],
                        channel_multiplier=-1,
                    )
                    nc.gpsimd.memset(mask01[t], 1.0)
                    nc.gpsimd.affine_select(
                        out=mask01[t][:],
                        in_=mask01[t][:],
                        compare_op=mybir.AluOpType.is_ge,
                        fill=0.0,
                        base=-128 * t,
                        pattern=[[1, 512]],
                        channel_multiplier=-1,
                    )

            def load_v_chunk(c, tiles_per_chunk):
                # v chunk c covers k-tiles [c*tpc, (c+1)*tpc)
                t0 = c * tiles_per_chunk
                t1 = min(nt, t0 + tiles_per_chunk)
                for h in range(hpc):
                    nc.gpsimd.dma_start(
                        out=vplus[h]
                        .rearrange("p (t x) -> p t x", x=hdim + 1)[:, t0:t1, 0:hdim],
                        in_=v[t0 * 128 : t1 * 128, h, :].rearrange(
                            "(t p) d -> p t d", p=128
                        ),
                    )

            # ---- staging: SWDGE cast-load + PE transpose ------------------
            # Super-chunks of 8 k-tiles. The PE transposes each staged
            # [128 seq, 128 (h d)] tile into a PSUM buffer borrowed from the
            # mm1 score pool (bitcast fp16), then one wide ACT/DVE copy moves
            # 8 transposed tiles into qT/kT. Staging for super-chunk c+1 is
            # emitted AFTER main-loop groups G=2c,2c+1 so the PE pipeline
            # never serializes behind the whole staging phase.
            schunk = 8
            nsc = nt // schunk  # 4 super-chunks
            identity = persist.tile([128, 128], fp16, tag="identity")
            from concourse.masks import make_identity

            make_identity(nc, identity[:])

            # memset the ones columns of vplus before any v data lands
            for h in range(hpc):
                nc.vector.memset(vplus[h], 1.0)

            with (
                tc.tile_pool(name="psum_s", bufs=3, space="PSUM") as psum_s_pool,
                tc.tile_pool(name="psum_o", bufs=1, space="PSUM") as psum_o_pool,
            ):
                copy_rot = [0]

                def stage_superchunk(c):
                    for src_t, dstT in ((k, kT), (q, qT)):
                        src_r = src_t.rearrange("(t p) h d -> p t (h d)", p=128)
                        st = ld_pool.tile([128, schunk * 128], fp16, tag="ldstage")
                        nc.gpsimd.dma_start(
                            out=st.rearrange("p (t x) -> p t x", x=128),
                            in_=src_r[:, c * schunk : (c + 1) * schunk, :],
                        )
                        tr = psum_s_pool.tile([128, 1024], f32, tag="ps", name="ps")
                        trv = tr.bitcast(fp16)  # [128, 2048] fp16 view
                        for tt in range(schunk):
                            nc.tensor.transpose(
                                trv[:, tt * 128 : (tt + 1) * 128],
                                st[:, tt * 128 : (tt + 1) * 128],
                                identity[:],
                            )
                        # one wide PSUM->SBUF copy per super-chunk, alternating
                        dst = dstT[:, c * 1024 : (c + 1) * 1024]
                        if copy_rot[0] % 2 == 0:
                            nc.scalar.copy(dst, trv[:, 0:1024])
                        else:
                            nc.vector.tensor_copy(dst, trv[:, 0:1024])
                        copy_rot[0] += 1

                def load_v_superchunk(c):
                    t0, t1 = c * schunk, (c + 1) * schunk
                    for h in range(hpc):
                        nc.gpsimd.dma_start(
                            out=vplus[h]
                            .rearrange("p (t x) -> p t x", x=hdim + 1)[:, t0:t1, 0:hdim],
                            in_=v[t0 * 128 : t1 * 128, h, :].rearrange(
                                "(t p) d -> p t d", p=128
                            ),
                        )

                loop = _MainLoop(
                    nc, mybir, ng, hdim, psum_s_pool, psum_o_pool, pexp_pool,
                    out_pool, small_pool, qT, kT, vplus, amask, mask01, o,
                    hpc, Exp,
                )
                for c in range(nsc):
                    stage_superchunk(c)
                    if c == 0:
                        build_masks()
                    load_v_superchunk(c)
                    loop.emit_group(2 * c)
                    loop.emit_group(2 * c + 1)
                loop.flush()
    if split_waits:
        _split_multi_waits(nc)
    return nc


def _split_multi_waits(nc):
    """Walrus's codegen accepts at most one sync-wait per instruction on
    this toolchain. Hoist extra waits into standalone single-wait NoOps on
    the same engine queue (same semantics: the sequencer stalls in order)."""
    import concourse.mybir as mybir

    nsplit = 0
    for blk in nc.m.functions[0].blocks:
        newl = []
        for ins in blk.instructions:
            si = getattr(ins, "sync_info", None)
            if si is not None and si.on_wait and len(si.on_wait) > 1:
                waits = list(si.on_wait)
                for w in waits[:-1]:
                    newl.append(
                        mybir.InstNoOp(
                            name=f"{ins.name}-wsplit{nsplit}",
                            sync_info=mybir.SyncInfo(on_wait=[w], on_update=[]),
                            bass_nofuse=True,
                            engine=ins.engine,
                            ins=[],
                            outs=[],
                        )
                    )
                    nsplit += 1
                ins.sync_info = mybir.SyncInfo(
                    on_wait=[waits[-1]], on_update=list(si.on_update or [])
                )
            newl.append(ins)
        blk.instructions = newl
    return nsplit


class _MainLoop:
    """Emits main-loop groups interleaved with staging.

    One iteration = one 128-wide k-block j for both heads.  mm2 for
    iteration g is deferred until after mm1 of iteration g+2 (the PE
    queue always holds independent work while ACT/DVE compute exp).
    """

    def __init__(self, nc, mybir, ng, hdim, psum_s_pool, psum_o_pool,
                 pexp_pool, out_pool, small_pool, qT, kT, vplus, amask,
                 mask01, o, hpc, Exp):
        self.nc = nc
        self.mybir = mybir
        self.ng = ng
        self.hdim = hdim
        self.psum_s_pool = psum_s_pool
        self.psum_o_pool = psum_o_pool
        self.pexp_pool = pexp_pool
        self.out_pool = out_pool
        self.small_pool = small_pool
        self.qT = qT
        self.kT = kT
        self.vplus = vplus
        self.amask = amask
        self.mask01 = mask01
        self.o = o
        self.hpc = hpc
        self.Exp = Exp
        self.f32 = mybir.dt.float32
        self.fp16 = mybir.dt.float16
        self.i16 = mybir.dt.int16
        self.add = mybir.AluOpType.add
        self.mult = mybir.AluOpType.mult
        self.Copy = mybir.ActivationFunctionType.Copy
        self.pending = []
        # greedy ACT/DVE balance counters (estimated busy ns)
        self.busy = {"act": 0.0, "dve": 0.0}
        self.ACT_LAT = 420.0
        self.DVE_LAT = 145.0

    # --- exp paths -----------------------------------------------------
    def exp_act_pair(self, pe, ps):
        nc = self.nc
        nc.scalar.activation(out=pe[:, 0:1024], in_=ps[:, 0:1024],
                             func=self.Exp, scale=SCALE)
        self.busy["act"] += 1024 * 0.833 + self.ACT_LAT

    def exp_dve_pair(self, pe, ps, t, q0):
        nc = self.nc
        if t >= 0:
            for h in range(2):
                nc.vector.scalar_tensor_tensor(
                    out=pe[:, h * 512 + q0 : (h + 1) * 512].bitcast(self.i16),
                    in0=ps[:, h * 512 + q0 : (h + 1) * 512],
                    scalar=EXP_BOA,
                    in1=self.amask[t][:, q0:512],
                    op0=self.add,
                    op1=self.mult,
                )
                self.busy["dve"] += (512 - q0) * 1.042 + self.DVE_LAT
        else:
            nc.vector.tensor_scalar(
                out=pe[:, 0:1024].bitcast(self.i16),
                in0=ps[:, 0:1024],
                scalar1=EXP_BOA,
                scalar2=EXP_A,
                op0=self.add,
                op1=self.mult,
            )
            self.busy["dve"] += 1024 * 1.042 + self.DVE_LAT

    # --- mm2 + finals --------------------------------------------------
    def emit_mm2(self, st):
        nc = self.nc
        G, j, po, pe, njs, last = st
        t = j - 4 * G
        hdim = self.hdim
        for h in range(self.hpc):
            for c in range(4):
                if t > c:
                    continue  # chunk fully masked -> zero contribution
                nc.tensor.matmul(
                    po[h][:, c * 128 : c * 128 + hdim + 1],
                    lhsT=pe[:, h * 512 + c * 128 : h * 512 + (c + 1) * 128],
                    rhs=self.vplus[h][:, j * 65 : j * 65 + hdim + 1],
                    start=(j == 0 and c == 0),
                    stop=(j == njs - 1 and c == 3),
                    skip_group_check=True,
                )

    def emit_finals(self, G, po):
        # Copy po out of PSUM immediately (frees the bank for the next G's
        # mm2 accumulation), then do reciprocal+normalize from SBUF so the
        # normalize can run on the otherwise-idle GPSIMD engine.
        nc = self.nc
        hdim = self.hdim
        pos = []
        for h in range(self.hpc):
            p_sb = self.out_pool.tile([128, 260], self.f32, tag="posb", name="posb")
            src_ap = po[h].rearrange("p (c x) -> p c x", x=128)[:, :, 0 : hdim + 1]
            dst_ap = p_sb.rearrange("p (c x) -> p c x", x=hdim + 1)
            if self.busy["act"] <= self.busy["dve"]:
                nc.scalar.copy(dst_ap, src_ap)
                self.busy["act"] += 260 * 0.833 + self.ACT_LAT
            else:
                nc.vector.tensor_copy(dst_ap, src_ap)
                self.busy["dve"] += 260 * 1.042 + self.DVE_LAT
            pos.append(p_sb)
        recs = []
        for h in range(self.hpc):
            rec4 = self.small_pool.tile([128, 4], self.f32, tag="rec4", name="rec4")
            nc.vector.reciprocal(
                rec4,
                pos[h].rearrange("p (c x) -> p c x", x=hdim + 1)[:, :, hdim : hdim + 1],
            )
            recs.append(rec4)
        ob = self.out_pool.tile([128, 4 * self.hpc * hdim], self.f32, tag="ob", name="ob")
        obv = ob.rearrange("p (c h d) -> p c h d", c=4, h=self.hpc)
        for h in range(self.hpc):
            # one batched normalize per head: broadcast rec4 over the 64
            # feature columns with a 0-stride AP
            rec_b = recs[h].broadcast_to([128, 4, hdim])
            nc.gpsimd.tensor_mul(
                obv[:, :, h, :],
                pos[h].rearrange("p (c x) -> p c x", x=hdim + 1)[:, :, 0:hdim],
                rec_b,
            )
        nc.sync.dma_start(
            out=self.o[G * 512 : (G + 1) * 512, :].rearrange("(c p) f -> p c f", p=128),
            in_=ob.rearrange("p (c f) -> p c f", c=4),
        )

    # --- per-group emission --------------------------------------------
    def emit_group(self, G):
        nc = self.nc
        njs = 4 * G + 4
        po = [
            self.psum_o_pool.tile([128, 512], self.f32, tag=f"po{h}", name=f"po{h}")
            for h in range(self.hpc)
        ]
        for j in range(njs):
            t = j - 4 * G
            ps = self.psum_s_pool.tile([128, 1024], self.f32, tag="ps", name="ps")
            q0 = 128 * t if (t > 0 and G >= 1) else 0
            for h in range(self.hpc):
                nc.tensor.matmul(
                    ps[:, h * 512 + q0 : (h + 1) * 512],
                    lhsT=self.kT[h * 64 : (h + 1) * 64, j * 128 : (j + 1) * 128],
                    rhs=self.qT[h * 64 : (h + 1) * 64, G * 512 + q0 : (G + 1) * 512],
                    start=True,
                    stop=True,
                    tile_position=(h * 64, 0),
                )
            pe = self.pexp_pool.tile([128, 1024], self.fp16, tag="pexp", name="pexp")
            if G == 0:
                # exact path with 0/1 mask multiplies (DVE, fp16 2x)
                self.exp_act_pair(pe, ps)
                for h in range(self.hpc):
                    nc.gpsimd.tensor_mul(
                        pe[:, h * 512 : (h + 1) * 512],
                        pe[:, h * 512 : (h + 1) * 512],
                        self.mask01[t][:],
                    )
            elif t >= 0:
                self.exp_dve_pair(pe, ps, t, q0)
            elif self.busy["act"] <= self.busy["dve"]:
                self.exp_act_pair(pe, ps)
            else:
                self.exp_dve_pair(pe, ps, t, 0)
            self.pending.append((G, j, po, pe, njs, j == njs - 1))
            if len(self.pending) > 2:
                st = self.pending.pop(0)
                self.emit_mm2(st)
                if st[5]:
                    self.emit_finals(st[0], st[2])

    def flush(self):
        for st in self.pending:
            self.emit_mm2(st)
            if st[5]:
                self.emit_finals(st[0], st[2])
        self.pending = []


def _ensure_ntff_hook():
    """The image's antenv package lacks axon_hooks; provide it so
    run_bass_kernel_spmd's trace path works (or degrades gracefully)."""
    import sys
    import types

    try:
        import antenv.axon_hooks  # noqa: F401

        return
    except ImportError:
        pass
    mod = types.ModuleType("antenv.axon_hooks")
    state = {"hook": None}
    mod.set_axon_ntff_profile_hook = lambda h: state.__setitem__("hook", h)
    mod.get_axon_ntff_profile_hook = lambda: state["hook"]
    try:
        from trn_agent_boot.trn_boot import _ntff_profile_via_ctypes

        state["hook"] = _ntff_profile_via_ctypes("/opt/axon/libaxon_pjrt.so")
    except Exception:
        state["hook"] = None
    sys.modules["antenv.axon_hooks"] = mod


def kernel(q, k, v):
    """Full-input entry point: q, k, v [4096, 16, 64] fp32 -> [4096, 1024]."""
    import sys

    if "/opt/trn_rl_repo" not in sys.path:
        sys.path.insert(0, "/opt/trn_rl_repo")
    _ensure_ntff_hook()
    from concourse.bass_utils import run_bass_kernel_spmd

    q = np.asarray(q, dtype=np.float32)
    k = np.asarray(k, dtype=np.float32)
    v = np.asarray(v, dtype=np.float32)
    seq, nhead, hdim = q.shape

    if "nc" not in _NC_CACHE:
        _NC_CACHE["nc"] = build_attention_nc(seq=seq, hpc=HPC, hdim=hdim)
    nc = _NC_CACHE["nc"]

    in_maps = []
    for c in range(NCORES):
        hs = slice(c * HPC, (c + 1) * HPC)
        in_maps.append(
            {
                "q": np.ascontiguousarray(q[:, hs, :]),
                "k": np.ascontiguousarray(k[:, hs, :]),
                "v": np.ascontiguousarray(v[:, hs, :]),
            }
        )
    res = run_bass_kernel_spmd(nc, in_maps, core_ids=list(range(NCORES)))
    LAST_RESULT["exec_time_ns"] = res.exec_time_ns
    try:
        iat = res.instructions_and_trace
        LAST_RESULT["trace_path"] = iat[1] if iat else None
    except Exception:
        LAST_RESULT["trace_path"] = None
    outs = [res.results[c]["o"] for c in range(NCORES)]
    return np.concatenate(outs, axis=1)


# revision 15
# speedup vs baseline: 2.1163x; 1.0702x over previous
"""Trainium2 Bass kernel for multi-head causal attention.

Problem: q, k, v of shape [4096, 16, 64] (seq, heads, head_dim) fp32.
  out = softmax(causal(q @ k^T / 8)) @ v, reshaped to [4096, 1024].

Sharding: heads are split across 8 NeuronCores (2 heads per core).
Each core runs the same SPMD Bass program on its own 2 heads; the host
concatenates the per-core [4096, 128] outputs along the feature dim.

Per-core algorithm (flash-attention style, S^T orientation), v2:
  - Stage Q, K as fp16 via SWDGE cast DMA into [128 seq, (h d)] tiles,
    then DMA XBAR-transpose (16x128 tiles, sync queue) into qT/kT
    [128=(h,d), 4096].  The PE does no staging work at all.
  - V per head into vplus [128, 32*65] fp16: 64 V columns plus a ones
    column per 128-row k-block (fused softmax denominator).
  - Main loop over (G, j): one 128-wide k-block j per iteration, both
    heads:
      mm1: S^T[kj, qi] for h0/h1 emitted back-to-back into one combined
           PSUM tile [128, 1024] with tile_position=(h*64, 0) so the two
           K=64 matmuls stream concurrently on disjoint PE row groups.
      exp: split across three engines.  ACT computes exact
           exp(s*0.125) -> fp16.  DVE / GPSIMD compute a Schraudolph
           approximation: t = (s + B/A)*A truncated to int16 and
           bitcast as fp16 equals 2^(s*0.125*log2 e) up to a constant
           factor (cancels in softmax) and a +-2% sawtooth.  For
           diagonal blocks the multiplier A is a precomputed per-element
           tensor (A where causally valid, 0 where masked) so masked
           lanes produce exactly +0.0.  G0 runs on the exact ACT path
           with 0/1 mask multiplies (small-denominator safety).
      mm2: O[qi, 64+1] += expS^T_chunk.T @ vplus_j, deferred two
           iterations (software pipelining keeps the PE queue full so
           the PE p-state can ramp to 2.4 GHz).
  - Normalize: batched reciprocal of the ones-columns (DVE), row-scale
    on GPSIMD, DMA out on the sync queue.
"""

import math

import numpy as np

SEQ = 4096
NHEAD = 16
HDIM = 64
NCORES = 8
HPC = NHEAD // NCORES  # heads per core = 2
SCALE = 0.125

# Schraudolph exp2 constants for fp16 bitcast output.
# t = (s + B/A) * A ; P = bitcast_fp16(int16(t)) ~= C * exp(s * SCALE)
EXP_A = 1024.0 / math.log(2.0) * SCALE  # 184.665
EXP_CORR = -0.0434  # sawtooth centering (constant factor cancels)
EXP_B = 15360.0 + EXP_CORR * 1024.0 + 0.5  # +0.5 centers the truncation
EXP_BOA = EXP_B / EXP_A

_NC_CACHE = {}
LAST_RESULT = {}


def build_attention_nc(seq=SEQ, hpc=HPC, hdim=HDIM, gp_exp=True, split_waits=True):
    """Build the SPMD Bass program for one core handling `hpc` heads."""
    import concourse.bass as bass
    import concourse.mybir as mybir
    import concourse.tile as tile

    f32 = mybir.dt.float32
    fp16 = mybir.dt.float16
    i16 = mybir.dt.int16
    Exp = mybir.ActivationFunctionType.Exp

    assert hpc == 2 and hdim == 64, "layout hardcoded for 2 heads x 64 dim"
    assert seq % 512 == 0
    nt = seq // 128   # number of 128-row seq tiles (32)
    ng = seq // 512   # number of 512-wide q groups (8)

    nc = bass.Bass()
    q = nc.dram_tensor("q", [seq, hpc, hdim], f32, kind="ExternalInput").ap()
    k = nc.dram_tensor("k", [seq, hpc, hdim], f32, kind="ExternalInput").ap()
    v = nc.dram_tensor("v", [seq, hpc, hdim], f32, kind="ExternalOutput" if False else "ExternalInput").ap()
    o = nc.dram_tensor("o", [seq, hpc * hdim], f32, kind="ExternalOutput").ap()

    with tile.TileContext(nc) as tc:
        with (
            tc.tile_pool(name="persist", bufs=1) as persist,
            tc.tile_pool(name="ldstage", bufs=4) as ld_pool,
            tc.tile_pool(name="pexp", bufs=4) as pexp_pool,
            tc.tile_pool(name="outp", bufs=6) as out_pool,
            tc.tile_pool(name="small", bufs=8) as small_pool,
        ):
            # ---- persistent SBUF tensors ----------------------------------
            qT = persist.tile([128, seq], fp16, tag="qT")
            kT = persist.tile([128, seq], fp16, tag="kT")
            vplus = [
                persist.tile([128, nt * (hdim + 1)], fp16, tag=f"vplus{h}", name=f"vplus{h}")
                for h in range(hpc)
            ]
            # amask_t[kj, qi] = EXP_A where kj + 128*t <= qi else 0.0
            # (fused causal mask for the Schraudolph path)
            amask = [persist.tile([128, 512], f32, tag=f"amask{t}", name=f"amask{t}") for t in range(4)]
            # mask01_t: 1/0 causal masks, fp16, for the G0 exact path.
            mask01 = [persist.tile([128, 512], fp16, tag=f"mask01_{t}", name=f"mask01_{t}") for t in range(4)]

            def build_masks():
                for t in range(4):
                    nc.gpsimd.memset(amask[t], EXP_A)
                    nc.gpsimd.affine_select(
                        out=amask[t][:],
                        in_=amask[t][:],
                        compare_op=mybir.AluOpType.is_ge,
                        fill=0.0,
                        base=-128 * t,
                        pattern=[[1, 512]],
                        channel_multiplier=-1,
                    )
                    nc.gpsimd.memset(mask01[t], 1.0)
                    nc.gpsimd.affine_select(
                        out=mask01[t][:],
                        in_=mask01[t][:],
                        compare_op=mybir.AluOpType.is_ge,
                        fill=0.0,
                        base=-128 * t,
                        pattern=[[1, 512]],
                        channel_multiplier=-1,
                    )

            def load_v_chunk(c, tiles_per_chunk):
                # v chunk c covers k-tiles [c*tpc, (c+1)*tpc)
                t0 = c * tiles_per_chunk
                t1 = min(nt, t0 + tiles_per_chunk)
                for h in range(hpc):
                    nc.gpsimd.dma_start(
                        out=vplus[h]
                        .rearrange("p (t x) -> p t x", x=hdim + 1)[:, t0:t1, 0:hdim],
                        in_=v[t0 * 128 : t1 * 128, h, :].rearrange(
                            "(t p) d -> p t d", p=128
                        ),
                    )

            # ---- staging: SWDGE cast-load + PE transpose ------------------
            # Super-chunks of 8 k-tiles. The PE transposes each staged
            # [128 seq, 128 (h d)] tile into a PSUM buffer borrowed from the
            # mm1 score pool (bitcast fp16), then one wide ACT/DVE copy moves
            # 8 transposed tiles into qT/kT. Staging for super-chunk c+1 is
            # emitted AFTER main-loop groups G=2c,2c+1 so the PE pipeline
            # never serializes behind the whole staging phase.
            schunk = 8
            nsc = nt // schunk  # 4 super-chunks
            identity = persist.tile([128, 128], fp16, tag="identity")
            from concourse.masks import make_identity

            make_identity(nc, identity[:])

            # memset the ones columns of vplus before any v data lands
            for h in range(hpc):
                nc.vector.memset(vplus[h], 1.0)

            with (
                tc.tile_pool(name="psum_s", bufs=3, space="PSUM") as psum_s_pool,
                tc.tile_pool(name="psum_o", bufs=1, space="PSUM") as psum_o_pool,
            ):
                copy_rot = [0]

                def stage_tiles(t0, ntile):
                    for src_t, dstT in ((k, kT), (q, qT)):
                        src_r = src_t.rearrange("(t p) h d -> p t (h d)", p=128)
                        st = ld_pool.tile([128, schunk * 128], fp16, tag="ldstage")
                        nc.gpsimd.dma_start(
                            out=st.rearrange("p (t x) -> p t x", x=128)[:, 0:ntile, :],
                            in_=src_r[:, t0 : t0 + ntile, :],
                        )
                        tr = psum_s_pool.tile([128, 1024], f32, tag="ps", name="ps")
                        trv = tr.bitcast(fp16)  # [128, 2048] fp16 view
                        for tt in range(ntile):
                            nc.tensor.transpose(
                                trv[:, tt * 128 : (tt + 1) * 128],
                                st[:, tt * 128 : (tt + 1) * 128],
                                identity[:],
                            )
                        # one wide PSUM->SBUF copy per chunk, alternating
                        dst = dstT[:, t0 * 128 : (t0 + ntile) * 128]
                        if copy_rot[0] % 2 == 0:
                            nc.scalar.copy(dst, trv[:, 0 : ntile * 128])
                        else:
                            nc.vector.tensor_copy(dst, trv[:, 0 : ntile * 128])
                        copy_rot[0] += 1

                def stage_superchunk(c):
                    if c == 0:
                        stage_tiles(0, 4)
                        stage_tiles(4, 4)
                    else:
                        stage_tiles(c * schunk, schunk)

                def load_v_superchunk(c):
                    t0, t1 = c * schunk, (c + 1) * schunk
                    for h in range(hpc):
                        nc.gpsimd.dma_start(
                            out=vplus[h]
                            .rearrange("p (t x) -> p t x", x=hdim + 1)[:, t0:t1, 0:hdim],
                            in_=v[t0 * 128 : t1 * 128, h, :].rearrange(
                                "(t p) d -> p t d", p=128
                            ),
                        )

                loop = _MainLoop(
                    nc, mybir, ng, hdim, psum_s_pool, psum_o_pool, pexp_pool,
                    out_pool, small_pool, qT, kT, vplus, amask, mask01, o,
                    hpc, Exp,
                )
                for c in range(nsc):
                    stage_superchunk(c)
                    if c == 0:
                        build_masks()
                    load_v_superchunk(c)
                    loop.emit_group(2 * c)
                    loop.emit_group(2 * c + 1)
                loop.flush()
    if split_waits:
        _split_multi_waits(nc)
    return nc


def _split_multi_waits(nc):
    """Walrus's codegen accepts at most one sync-wait per instruction on
    this toolchain. Hoist extra waits into standalone single-wait NoOps on
    the same engine queue (same semantics: the sequencer stalls in order)."""
    import concourse.mybir as mybir

    nsplit = 0
    for blk in nc.m.functions[0].blocks:
        newl = []
        for ins in blk.instructions:
            si = getattr(ins, "sync_info", None)
            if si is not None and si.on_wait and len(si.on_wait) > 1:
                waits = list(si.on_wait)
                for w in waits[:-1]:
                    newl.append(
                        mybir.InstNoOp(
                            name=f"{ins.name}-wsplit{nsplit}",
                            sync_info=mybir.SyncInfo(on_wait=[w], on_update=[]),
                            bass_nofuse=True,
                            engine=ins.engine,
                            ins=[],
                            outs=[],
                        )
                    )
                    nsplit += 1
                ins.sync_info = mybir.SyncInfo(
                    on_wait=[waits[-1]], on_update=list(si.on_update or [])
                )
            newl.append(ins)
        blk.instructions = newl
    return nsplit


class _MainLoop:
    """Emits main-loop groups interleaved with staging.

    One iteration = one 128-wide k-block j for both heads.  mm2 for
    iteration g is deferred until after mm1 of iteration g+2 (the PE
    queue always holds independent work while ACT/DVE compute exp).
    """

    def __init__(self, nc, mybir, ng, hdim, psum_s_pool, psum_o_pool,
                 pexp_pool, out_pool, small_pool, qT, kT, vplus, amask,
                 mask01, o, hpc, Exp):
        self.nc = nc
        self.mybir = mybir
        self.ng = ng
        self.hdim = hdim
        self.psum_s_pool = psum_s_pool
        self.psum_o_pool = psum_o_pool
        self.pexp_pool = pexp_pool
        self.out_pool = out_pool
        self.small_pool = small_pool
        self.qT = qT
        self.kT = kT
        self.vplus = vplus
        self.amask = amask
        self.mask01 = mask01
        self.o = o
        self.hpc = hpc
        self.Exp = Exp
        self.f32 = mybir.dt.float32
        self.fp16 = mybir.dt.float16
        self.i16 = mybir.dt.int16
        self.add = mybir.AluOpType.add
        self.mult = mybir.AluOpType.mult
        self.Copy = mybir.ActivationFunctionType.Copy
        self.pending = []
        # greedy ACT/DVE balance counters (estimated busy ns)
        self.busy = {"act": 0.0, "dve": 0.0}
        self.ACT_LAT = 280.0
        self.DVE_LAT = 145.0

    # --- exp paths -----------------------------------------------------
    def exp_act_pair(self, pe, ps):
        nc = self.nc
        nc.scalar.activation(out=pe[:, 0:1024], in_=ps[:, 0:1024],
                             func=self.Exp, scale=SCALE)
        self.busy["act"] += 1024 * 0.833 + self.ACT_LAT

    def exp_dve_pair(self, pe, ps, t, q0):
        nc = self.nc
        if t >= 0:
            w = 512 - q0
            nc.vector.scalar_tensor_tensor(
                out=pe.rearrange("p (h x) -> p h x", h=2)[:, :, q0:512].bitcast(self.i16),
                in0=ps.rearrange("p (h x) -> p h x", h=2)[:, :, q0:512],
                scalar=EXP_BOA,
                in1=self.amask[t][:, q0:512].rearrange("p x -> p () x").broadcast_to([128, 2, w]),
                op0=self.add,
                op1=self.mult,
            )
            self.busy["dve"] += 2 * w * 1.042 + self.DVE_LAT
        else:
            nc.vector.tensor_scalar(
                out=pe[:, 0:1024].bitcast(self.i16),
                in0=ps[:, 0:1024],
                scalar1=EXP_BOA,
                scalar2=EXP_A,
                op0=self.add,
                op1=self.mult,
            )
            self.busy["dve"] += 1024 * 1.042 + self.DVE_LAT

    # --- mm2 + finals --------------------------------------------------
    def emit_mm2(self, st):
        nc = self.nc
        G, j, po, pe, njs, last = st
        t = j - 4 * G
        hdim = self.hdim
        for h in range(self.hpc):
            for c in range(4):
                if t > c:
                    continue  # chunk fully masked -> zero contribution
                nc.tensor.matmul(
                    po[h][:, c * 128 : c * 128 + hdim + 1],
                    lhsT=pe[:, h * 512 + c * 128 : h * 512 + (c + 1) * 128],
                    rhs=self.vplus[h][:, j * 65 : j * 65 + hdim + 1],
                    start=(j == 0 and c == 0),
                    stop=(j == njs - 1 and c == 3),
                    skip_group_check=True,
                )

    def emit_finals(self, G, po):
        # Copy po out of PSUM immediately (frees the bank for the next G's
        # mm2 accumulation), then do reciprocal+normalize from SBUF so the
        # normalize can run on the otherwise-idle GPSIMD engine.
        nc = self.nc
        hdim = self.hdim
        pos = []
        for h in range(self.hpc):
            p_sb = self.out_pool.tile([128, 260], self.f32, tag="posb", name="posb")
            src_ap = po[h].rearrange("p (c x) -> p c x", x=128)[:, :, 0 : hdim + 1]
            dst_ap = p_sb.rearrange("p (c x) -> p c x", x=hdim + 1)
            if self.busy["act"] <= self.busy["dve"]:
                nc.scalar.copy(dst_ap, src_ap)
                self.busy["act"] += 260 * 0.833 + self.ACT_LAT
            else:
                nc.vector.tensor_copy(dst_ap, src_ap)
                self.busy["dve"] += 260 * 1.042 + self.DVE_LAT
            pos.append(p_sb)
        recs = []
        for h in range(self.hpc):
            rec4 = self.small_pool.tile([128, 4], self.f32, tag="rec4", name="rec4")
            nc.vector.reciprocal(
                rec4,
                pos[h].rearrange("p (c x) -> p c x", x=hdim + 1)[:, :, hdim : hdim + 1],
            )
            recs.append(rec4)
        ob = self.out_pool.tile([128, 4 * self.hpc * hdim], self.f32, tag="ob", name="ob")
        obv = ob.rearrange("p (c h d) -> p c h d", c=4, h=self.hpc)
        for h in range(self.hpc):
            # one batched normalize per head: broadcast rec4 over the 64
            # feature columns with a 0-stride AP
            rec_b = recs[h].broadcast_to([128, 4, hdim])
            nc.gpsimd.tensor_mul(
                obv[:, :, h, :],
                pos[h].rearrange("p (c x) -> p c x", x=hdim + 1)[:, :, 0:hdim],
                rec_b,
            )
        nc.sync.dma_start(
            out=self.o[G * 512 : (G + 1) * 512, :].rearrange("(c p) f -> p c f", p=128),
            in_=ob.rearrange("p (c f) -> p c f", c=4),
        )

    # --- per-group emission --------------------------------------------
    def emit_group(self, G):
        nc = self.nc
        njs = 4 * G + 4
        po = [
            self.psum_o_pool.tile([128, 512], self.f32, tag=f"po{h}", name=f"po{h}")
            for h in range(self.hpc)
        ]
        for j in range(njs):
            t = j - 4 * G
            ps = self.psum_s_pool.tile([128, 1024], self.f32, tag="ps", name="ps")
            q0 = 128 * t if (t > 0 and G >= 1) else 0
            for h in range(self.hpc):
                nc.tensor.matmul(
                    ps[:, h * 512 + q0 : (h + 1) * 512],
                    lhsT=self.kT[h * 64 : (h + 1) * 64, j * 128 : (j + 1) * 128],
                    rhs=self.qT[h * 64 : (h + 1) * 64, G * 512 + q0 : (G + 1) * 512],
                    start=True,
                    stop=True,
                    tile_position=(h * 64, 0),
                )
            pe = self.pexp_pool.tile([128, 1024], self.fp16, tag="pexp", name="pexp")
            if G == 0:
                # exact path with 0/1 mask multiplies (DVE, fp16 2x)
                self.exp_act_pair(pe, ps)
                for h in range(self.hpc):
                    nc.gpsimd.tensor_mul(
                        pe[:, h * 512 : (h + 1) * 512],
                        pe[:, h * 512 : (h + 1) * 512],
                        self.mask01[t][:],
                    )
            elif t >= 0:
                self.exp_dve_pair(pe, ps, t, q0)
            elif self.busy["act"] <= self.busy["dve"]:
                self.exp_act_pair(pe, ps)
            else:
                self.exp_dve_pair(pe, ps, t, 0)
            self.pending.append((G, j, po, pe, njs, j == njs - 1))
            if len(self.pending) > 3:
                st = self.pending.pop(0)
                self.emit_mm2(st)
                if st[5]:
                    self.emit_finals(st[0], st[2])

    def flush(self):
        for st in self.pending:
            self.emit_mm2(st)
            if st[5]:
                self.emit_finals(st[0], st[2])
        self.pending = []


def _ensure_ntff_hook():
    """The image's antenv package lacks axon_hooks; provide it so
    run_bass_kernel_spmd's trace path works (or degrades gracefully)."""
    import sys
    import types

    try:
        import antenv.axon_hooks  # noqa: F401

        return
    except ImportError:
        pass
    mod = types.ModuleType("antenv.axon_hooks")
    state = {"hook": None}
    mod.set_axon_ntff_profile_hook = lambda h: state.__setitem__("hook", h)
    mod.get_axon_ntff_profile_hook = lambda: state["hook"]
    try:
        from trn_agent_boot.trn_boot import _ntff_profile_via_ctypes

        state["hook"] = _ntff_profile_via_ctypes("/opt/axon/libaxon_pjrt.so")
    except Exception:
        state["hook"] = None
    sys.modules["antenv.axon_hooks"] = mod


def kernel(q, k, v):
    """Full-input entry point: q, k, v [4096, 16, 64] fp32 -> [4096, 1024]."""
    import sys

    if "/opt/trn_rl_repo" not in sys.path:
        sys.path.insert(0, "/opt/trn_rl_repo")
    _ensure_ntff_hook()
    from concourse.bass_utils import run_bass_kernel_spmd

    q = np.asarray(q, dtype=np.float32)
    k = np.asarray(k, dtype=np.float32)
    v = np.asarray(v, dtype=np.float32)
    seq, nhead, hdim = q.shape

    if "nc" not in _NC_CACHE:
        _NC_CACHE["nc"] = build_attention_nc(seq=seq, hpc=HPC, hdim=hdim)
    nc = _NC_CACHE["nc"]

    in_maps = []
    for c in range(NCORES):
        hs = slice(c * HPC, (c + 1) * HPC)
        in_maps.append(
            {
                "q": np.ascontiguousarray(q[:, hs, :]),
                "k": np.ascontiguousarray(k[:, hs, :]),
                "v": np.ascontiguousarray(v[:, hs, :]),
            }
        )
    res = run_bass_kernel_spmd(nc, in_maps, core_ids=list(range(NCORES)))
    LAST_RESULT["exec_time_ns"] = res.exec_time_ns
    try:
        iat = res.instructions_and_trace
        LAST_RESULT["trace_path"] = iat[1] if iat else None
    except Exception:
        LAST_RESULT["trace_path"] = None
    outs = [res.results[c]["o"] for c in range(NCORES)]
    return np.concatenate(outs, axis=1)


# revision 17
# speedup vs baseline: 2.2234x; 1.0506x over previous
"""Trainium2 Bass kernel for multi-head causal attention.

Problem: q, k, v of shape [4096, 16, 64] (seq, heads, head_dim) fp32.
  out = softmax(causal(q @ k^T / 8)) @ v, reshaped to [4096, 1024].

Sharding: heads are split across 8 NeuronCores (2 heads per core).
Each core runs the same SPMD Bass program on its own 2 heads; the host
concatenates the per-core [4096, 128] outputs along the feature dim.

Per-core algorithm (flash-attention style, S^T orientation), v2:
  - Stage Q, K as fp16 via SWDGE cast DMA into [128 seq, (h d)] tiles,
    then DMA XBAR-transpose (16x128 tiles, sync queue) into qT/kT
    [128=(h,d), 4096].  The PE does no staging work at all.
  - V per head into vplus [128, 32*65] fp16: 64 V columns plus a ones
    column per 128-row k-block (fused softmax denominator).
  - Main loop over (G, j): one 128-wide k-block j per iteration, both
    heads:
      mm1: S^T[kj, qi] for h0/h1 emitted back-to-back into one combined
           PSUM tile [128, 1024] with tile_position=(h*64, 0) so the two
           K=64 matmuls stream concurrently on disjoint PE row groups.
      exp: split across three engines.  ACT computes exact
           exp(s*0.125) -> fp16.  DVE / GPSIMD compute a Schraudolph
           approximation: t = (s + B/A)*A truncated to int16 and
           bitcast as fp16 equals 2^(s*0.125*log2 e) up to a constant
           factor (cancels in softmax) and a +-2% sawtooth.  For
           diagonal blocks the multiplier A is a precomputed per-element
           tensor (A where causally valid, 0 where masked) so masked
           lanes produce exactly +0.0.  G0 runs on the exact ACT path
           with 0/1 mask multiplies (small-denominator safety).
      mm2: O[qi, 64+1] += expS^T_chunk.T @ vplus_j, deferred two
           iterations (software pipelining keeps the PE queue full so
           the PE p-state can ramp to 2.4 GHz).
  - Normalize: batched reciprocal of the ones-columns (DVE), row-scale
    on GPSIMD, DMA out on the sync queue.
"""

import math

import numpy as np

SEQ = 4096
NHEAD = 16
HDIM = 64
NCORES = 8
HPC = NHEAD // NCORES  # heads per core = 2
SCALE = 0.125

# Schraudolph exp2 constants for fp16 bitcast output.
# t = (s + B/A) * A ; P = bitcast_fp16(int16(t)) ~= C * exp(s * SCALE)
EXP_A = 1024.0 / math.log(2.0) * SCALE  # 184.665
EXP_CORR = -0.0434  # sawtooth centering (constant factor cancels)
EXP_B = 15360.0 + EXP_CORR * 1024.0 + 0.5  # +0.5 centers the truncation
EXP_BOA = EXP_B / EXP_A

_NC_CACHE = {}
LAST_RESULT = {}


def build_attention_nc(seq=SEQ, hpc=HPC, hdim=HDIM, gp_exp=True, split_waits=True):
    """Build the SPMD Bass program for one core handling `hpc` heads."""
    import concourse.bass as bass
    import concourse.mybir as mybir
    import concourse.tile as tile

    f32 = mybir.dt.float32
    fp16 = mybir.dt.float16
    i16 = mybir.dt.int16
    Exp = mybir.ActivationFunctionType.Exp

    assert hpc == 2 and hdim == 64, "layout hardcoded for 2 heads x 64 dim"
    assert seq % 512 == 0
    nt = seq // 128   # number of 128-row seq tiles (32)
    ng = seq // 512   # number of 512-wide q groups (8)

    nc = bass.Bass()
    q = nc.dram_tensor("q", [seq, hpc, hdim], f32, kind="ExternalInput").ap()
    k = nc.dram_tensor("k", [seq, hpc, hdim], f32, kind="ExternalInput").ap()
    v = nc.dram_tensor("v", [seq, hpc, hdim], f32, kind="ExternalOutput" if False else "ExternalInput").ap()
    o = nc.dram_tensor("o", [seq, hpc * hdim], f32, kind="ExternalOutput").ap()

    with tile.TileContext(nc) as tc:
        with (
            tc.tile_pool(name="persist", bufs=1) as persist,
            tc.tile_pool(name="ldstage", bufs=4) as ld_pool,
            tc.tile_pool(name="pexp", bufs=4) as pexp_pool,
            tc.tile_pool(name="outp", bufs=6) as out_pool,
            tc.tile_pool(name="small", bufs=8) as small_pool,
        ):
            # ---- persistent SBUF tensors ----------------------------------
            qT = persist.tile([128, seq], fp16, tag="qT")
            kT = persist.tile([128, seq], fp16, tag="kT")
            vplus = [
                persist.tile([128, nt * (hdim + 1)], fp16, tag=f"vplus{h}", name=f"vplus{h}")
                for h in range(hpc)
            ]
            # amask_t[kj, qi] = EXP_A where kj + 128*t <= qi else 0.0
            # (fused causal mask for the Schraudolph path)
            amask = [persist.tile([128, 512], f32, tag=f"amask{t}", name=f"amask{t}") for t in range(4)]
            # mask01_t: 1/0 causal masks, fp16, for the G0 exact path.
            mask01 = [persist.tile([128, 512], fp16, tag=f"mask01_{t}", name=f"mask01_{t}") for t in range(4)]

            def build_mask01(t):
                nc.vector.memset(mask01[t], 1.0)
                nc.gpsimd.affine_select(
                    out=mask01[t][:],
                    in_=mask01[t][:],
                    compare_op=mybir.AluOpType.is_ge,
                    fill=0.0,
                    base=-128 * t,
                    pattern=[[1, 512]],
                    channel_multiplier=-1,
                )

            def build_amasks():
                for t in range(4):
                    nc.vector.memset(amask[t], EXP_A)
                    nc.gpsimd.affine_select(
                        out=amask[t][:],
                        in_=amask[t][:],
                        compare_op=mybir.AluOpType.is_ge,
                        fill=0.0,
                        base=-128 * t,
                        pattern=[[1, 512]],
                        channel_multiplier=-1,
                    )

            def load_v_chunk(c, tiles_per_chunk):
                # v chunk c covers k-tiles [c*tpc, (c+1)*tpc)
                t0 = c * tiles_per_chunk
                t1 = min(nt, t0 + tiles_per_chunk)
                for h in range(hpc):
                    nc.gpsimd.dma_start(
                        out=vplus[h]
                        .rearrange("p (t x) -> p t x", x=hdim + 1)[:, t0:t1, 0:hdim],
                        in_=v[t0 * 128 : t1 * 128, h, :].rearrange(
                            "(t p) d -> p t d", p=128
                        ),
                    )

            # ---- staging: SWDGE cast-load + PE transpose ------------------
            # Super-chunks of 8 k-tiles. The PE transposes each staged
            # [128 seq, 128 (h d)] tile into a PSUM buffer borrowed from the
            # mm1 score pool (bitcast fp16), then one wide ACT/DVE copy moves
            # 8 transposed tiles into qT/kT. Staging for super-chunk c+1 is
            # emitted AFTER main-loop groups G=2c,2c+1 so the PE pipeline
            # never serializes behind the whole staging phase.
            schunk = 8
            nsc = nt // schunk  # 4 super-chunks
            identity = persist.tile([128, 128], fp16, tag="identity")
            from concourse.masks import make_identity

            make_identity(nc, identity[:])

            # memset the ones columns of vplus before any v data lands
            for h in range(hpc):
                nc.vector.memset(vplus[h], 1.0)

            with (
                tc.tile_pool(name="psum_s", bufs=3, space="PSUM") as psum_s_pool,
                tc.tile_pool(name="psum_o", bufs=1, space="PSUM") as psum_o_pool,
            ):
                copy_rot = [0]

                def stage_tiles(t0, ntile):
                    for src_t, dstT in ((k, kT), (q, qT)):
                        src_r = src_t.rearrange("(t p) h d -> p t (h d)", p=128)
                        st = ld_pool.tile([128, schunk * 128], fp16, tag="ldstage")
                        nc.gpsimd.dma_start(
                            out=st.rearrange("p (t x) -> p t x", x=128)[:, 0:ntile, :],
                            in_=src_r[:, t0 : t0 + ntile, :],
                        )
                        tr = psum_s_pool.tile([128, 1024], f32, tag="ps", name="ps")
                        trv = tr.bitcast(fp16)  # [128, 2048] fp16 view
                        for tt in range(ntile):
                            nc.tensor.transpose(
                                trv[:, tt * 128 : (tt + 1) * 128],
                                st[:, tt * 128 : (tt + 1) * 128],
                                identity[:],
                            )
                        # one wide PSUM->SBUF copy per chunk, alternating
                        dst = dstT[:, t0 * 128 : (t0 + ntile) * 128]
                        if copy_rot[0] % 2 == 0:
                            nc.scalar.copy(dst, trv[:, 0 : ntile * 128])
                        else:
                            nc.vector.tensor_copy(dst, trv[:, 0 : ntile * 128])
                        copy_rot[0] += 1

                def stage_superchunk(c):
                    if c == 0:
                        stage_tiles(0, 4)
                        for t in range(4):
                            build_mask01(t)
                        load_v_tiles(0, 4)
                        stage_tiles(4, 4)
                        load_v_tiles(4, 8)
                        build_amasks()
                    else:
                        stage_tiles(c * schunk, schunk)
                        load_v_tiles(c * schunk, (c + 1) * schunk)

                def load_v_tiles(t0, t1):
                    for h in range(hpc):
                        nc.gpsimd.dma_start(
                            out=vplus[h]
                            .rearrange("p (t x) -> p t x", x=hdim + 1)[:, t0:t1, 0:hdim],
                            in_=v[t0 * 128 : t1 * 128, h, :].rearrange(
                                "(t p) d -> p t d", p=128
                            ),
                        )

                loop = _MainLoop(
                    nc, mybir, ng, hdim, psum_s_pool, psum_o_pool, pexp_pool,
                    out_pool, small_pool, qT, kT, vplus, amask, mask01, o,
                    hpc, Exp,
                )
                for c in range(nsc):
                    stage_superchunk(c)
                    loop.emit_group(2 * c)
                    loop.emit_group(2 * c + 1)
                loop.flush()
    if split_waits:
        _split_multi_waits(nc)
    return nc


def _split_multi_waits(nc):
    """Walrus's codegen accepts at most one sync-wait per instruction on
    this toolchain. Hoist extra waits into standalone single-wait NoOps on
    the same engine queue (same semantics: the sequencer stalls in order)."""
    import concourse.mybir as mybir

    nsplit = 0
    for blk in nc.m.functions[0].blocks:
        newl = []
        for ins in blk.instructions:
            si = getattr(ins, "sync_info", None)
            if si is not None and si.on_wait and len(si.on_wait) > 1:
                waits = list(si.on_wait)
                for w in waits[:-1]:
                    newl.append(
                        mybir.InstNoOp(
                            name=f"{ins.name}-wsplit{nsplit}",
                            sync_info=mybir.SyncInfo(on_wait=[w], on_update=[]),
                            bass_nofuse=True,
                            engine=ins.engine,
                            ins=[],
                            outs=[],
                        )
                    )
                    nsplit += 1
                ins.sync_info = mybir.SyncInfo(
                    on_wait=[waits[-1]], on_update=list(si.on_update or [])
                )
            newl.append(ins)
        blk.instructions = newl
    return nsplit


class _MainLoop:
    """Emits main-loop groups interleaved with staging.

    One iteration = one 128-wide k-block j for both heads.  mm2 for
    iteration g is deferred until after mm1 of iteration g+2 (the PE
    queue always holds independent work while ACT/DVE compute exp).
    """

    def __init__(self, nc, mybir, ng, hdim, psum_s_pool, psum_o_pool,
                 pexp_pool, out_pool, small_pool, qT, kT, vplus, amask,
                 mask01, o, hpc, Exp):
        self.nc = nc
        self.mybir = mybir
        self.ng = ng
        self.hdim = hdim
        self.psum_s_pool = psum_s_pool
        self.psum_o_pool = psum_o_pool
        self.pexp_pool = pexp_pool
        self.out_pool = out_pool
        self.small_pool = small_pool
        self.qT = qT
        self.kT = kT
        self.vplus = vplus
        self.amask = amask
        self.mask01 = mask01
        self.o = o
        self.hpc = hpc
        self.Exp = Exp
        self.f32 = mybir.dt.float32
        self.fp16 = mybir.dt.float16
        self.i16 = mybir.dt.int16
        self.add = mybir.AluOpType.add
        self.mult = mybir.AluOpType.mult
        self.Copy = mybir.ActivationFunctionType.Copy
        self.pending = []
        # greedy ACT/DVE balance counters (estimated busy ns)
        self.busy = {"act": 0.0, "dve": 0.0}
        self.ACT_LAT = 280.0
        self.DVE_LAT = 145.0

    # --- exp paths -----------------------------------------------------
    def exp_act_pair(self, pe, ps):
        nc = self.nc
        nc.scalar.activation(out=pe[:, 0:1024], in_=ps[:, 0:1024],
                             func=self.Exp, scale=SCALE)
        self.busy["act"] += 1024 * 0.833 + self.ACT_LAT

    def exp_dve_pair(self, pe, ps, t, q0):
        nc = self.nc
        if t >= 0:
            w = 512 - q0
            nc.vector.scalar_tensor_tensor(
                out=pe.rearrange("p (h x) -> p h x", h=2)[:, :, q0:512].bitcast(self.i16),
                in0=ps.rearrange("p (h x) -> p h x", h=2)[:, :, q0:512],
                scalar=EXP_BOA,
                in1=self.amask[t][:, q0:512].rearrange("p x -> p () x").broadcast_to([128, 2, w]),
                op0=self.add,
                op1=self.mult,
            )
            self.busy["dve"] += 2 * w * 1.042 + self.DVE_LAT
        else:
            nc.vector.tensor_scalar(
                out=pe[:, 0:1024].bitcast(self.i16),
                in0=ps[:, 0:1024],
                scalar1=EXP_BOA,
                scalar2=EXP_A,
                op0=self.add,
                op1=self.mult,
            )
            self.busy["dve"] += 1024 * 1.042 + self.DVE_LAT

    # --- mm2 + finals --------------------------------------------------
    def emit_mm2(self, st):
        nc = self.nc
        G, j, po, pe, njs, last = st
        t = j - 4 * G
        hdim = self.hdim
        for h in range(self.hpc):
            for c in range(4):
                if t > c:
                    continue  # chunk fully masked -> zero contribution
                nc.tensor.matmul(
                    po[h][:, c * 128 : c * 128 + hdim + 1],
                    lhsT=pe[:, h * 512 + c * 128 : h * 512 + (c + 1) * 128],
                    rhs=self.vplus[h][:, j * 65 : j * 65 + hdim + 1],
                    start=(j == 0 and c == 0),
                    stop=(j == njs - 1 and c == 3),
                    skip_group_check=True,
                )

    def emit_finals(self, G, po):
        # Copy po out of PSUM immediately (frees the bank for the next G's
        # mm2 accumulation), then do reciprocal+normalize from SBUF so the
        # normalize can run on the otherwise-idle GPSIMD engine.
        nc = self.nc
        hdim = self.hdim
        pos = []
        for h in range(self.hpc):
            p_sb = self.out_pool.tile([128, 260], self.f32, tag="posb", name="posb")
            src_ap = po[h].rearrange("p (c x) -> p c x", x=128)[:, :, 0 : hdim + 1]
            dst_ap = p_sb.rearrange("p (c x) -> p c x", x=hdim + 1)
            if self.busy["act"] <= self.busy["dve"]:
                nc.scalar.copy(dst_ap, src_ap)
                self.busy["act"] += 260 * 0.833 + self.ACT_LAT
            else:
                nc.vector.tensor_copy(dst_ap, src_ap)
                self.busy["dve"] += 260 * 1.042 + self.DVE_LAT
            pos.append(p_sb)
        recs = []
        for h in range(self.hpc):
            rec4 = self.small_pool.tile([128, 4], self.f32, tag="rec4", name="rec4")
            nc.vector.reciprocal(
                rec4,
                pos[h].rearrange("p (c x) -> p c x", x=hdim + 1)[:, :, hdim : hdim + 1],
            )
            recs.append(rec4)
        ob = self.out_pool.tile([128, 4 * self.hpc * hdim], self.f32, tag="ob", name="ob")
        obv = ob.rearrange("p (c h d) -> p c h d", c=4, h=self.hpc)
        for h in range(self.hpc):
            # one batched normalize per head: broadcast rec4 over the 64
            # feature columns with a 0-stride AP
            rec_b = recs[h].broadcast_to([128, 4, hdim])
            nc.gpsimd.tensor_mul(
                obv[:, :, h, :],
                pos[h].rearrange("p (c x) -> p c x", x=hdim + 1)[:, :, 0:hdim],
                rec_b,
            )
        nc.sync.dma_start(
            out=self.o[G * 512 : (G + 1) * 512, :].rearrange("(c p) f -> p c f", p=128),
            in_=ob.rearrange("p (c f) -> p c f", c=4),
        )

    # --- per-group emission --------------------------------------------
    def emit_group(self, G):
        nc = self.nc
        njs = 4 * G + 4
        po = [
            self.psum_o_pool.tile([128, 512], self.f32, tag=f"po{h}", name=f"po{h}")
            for h in range(self.hpc)
        ]
        for j in range(njs):
            t = j - 4 * G
            ps = self.psum_s_pool.tile([128, 1024], self.f32, tag="ps", name="ps")
            q0 = 128 * t if (t > 0 and G >= 1) else 0
            for h in range(self.hpc):
                nc.tensor.matmul(
                    ps[:, h * 512 + q0 : (h + 1) * 512],
                    lhsT=self.kT[h * 64 : (h + 1) * 64, j * 128 : (j + 1) * 128],
                    rhs=self.qT[h * 64 : (h + 1) * 64, G * 512 + q0 : (G + 1) * 512],
                    start=True,
                    stop=True,
                    tile_position=(h * 64, 0),
                )
            pe = self.pexp_pool.tile([128, 1024], self.fp16, tag="pexp", name="pexp")
            if G == 0:
                # exact path with 0/1 mask multiplies (DVE, fp16 2x)
                self.exp_act_pair(pe, ps)
                for h in range(self.hpc):
                    nc.vector.tensor_mul(
                        pe[:, h * 512 : (h + 1) * 512],
                        pe[:, h * 512 : (h + 1) * 512],
                        self.mask01[t][:],
                    )
                    self.busy["dve"] += 512 * 0.521 + self.DVE_LAT
            elif t >= 0:
                self.exp_dve_pair(pe, ps, t, q0)
            elif self.busy["act"] <= self.busy["dve"]:
                self.exp_act_pair(pe, ps)
            else:
                self.exp_dve_pair(pe, ps, t, 0)
            self.pending.append((G, j, po, pe, njs, j == njs - 1))
            if len(self.pending) > 3:
                st = self.pending.pop(0)
                self.emit_mm2(st)
                if st[5]:
                    self.emit_finals(st[0], st[2])

    def flush(self):
        for st in self.pending:
            self.emit_mm2(st)
            if st[5]:
                self.emit_finals(st[0], st[2])
        self.pending = []


def _ensure_ntff_hook():
    """The image's antenv package lacks axon_hooks; provide it so
    run_bass_kernel_spmd's trace path works (or degrades gracefully)."""
    import sys
    import types

    try:
        import antenv.axon_hooks  # noqa: F401

        return
    except ImportError:
        pass
    mod = types.ModuleType("antenv.axon_hooks")
    state = {"hook": None}
    mod.set_axon_ntff_profile_hook = lambda h: state.__setitem__("hook", h)
    mod.get_axon_ntff_profile_hook = lambda: state["hook"]
    try:
        from trn_agent_boot.trn_boot import _ntff_profile_via_ctypes

        state["hook"] = _ntff_profile_via_ctypes("/opt/axon/libaxon_pjrt.so")
    except Exception:
        state["hook"] = None
    sys.modules["antenv.axon_hooks"] = mod


def kernel(q, k, v):
    """Full-input entry point: q, k, v [4096, 16, 64] fp32 -> [4096, 1024]."""
    import sys

    if "/opt/trn_rl_repo" not in sys.path:
        sys.path.insert(0, "/opt/trn_rl_repo")
    _ensure_ntff_hook()
    from concourse.bass_utils import run_bass_kernel_spmd

    q = np.asarray(q, dtype=np.float32)
    k = np.asarray(k, dtype=np.float32)
    v = np.asarray(v, dtype=np.float32)
    seq, nhead, hdim = q.shape

    if "nc" not in _NC_CACHE:
        _NC_CACHE["nc"] = build_attention_nc(seq=seq, hpc=HPC, hdim=hdim)
    nc = _NC_CACHE["nc"]

    in_maps = []
    for c in range(NCORES):
        hs = slice(c * HPC, (c + 1) * HPC)
        in_maps.append(
            {
                "q": np.ascontiguousarray(q[:, hs, :]),
                "k": np.ascontiguousarray(k[:, hs, :]),
                "v": np.ascontiguousarray(v[:, hs, :]),
            }
        )
    res = run_bass_kernel_spmd(nc, in_maps, core_ids=list(range(NCORES)))
    LAST_RESULT["exec_time_ns"] = res.exec_time_ns
    try:
        iat = res.instructions_and_trace
        LAST_RESULT["trace_path"] = iat[1] if iat else None
    except Exception:
        LAST_RESULT["trace_path"] = None
    outs = [res.results[c]["o"] for c in range(NCORES)]
    return np.concatenate(outs, axis=1)


# revision 18
# speedup vs baseline: 2.3001x; 1.0345x over previous
"""Trainium2 Bass kernel for multi-head causal attention.

Problem: q, k, v of shape [4096, 16, 64] (seq, heads, head_dim) fp32.
  out = softmax(causal(q @ k^T / 8)) @ v, reshaped to [4096, 1024].

Sharding: heads are split across 8 NeuronCores (2 heads per core).
Each core runs the same SPMD Bass program on its own 2 heads; the host
concatenates the per-core [4096, 128] outputs along the feature dim.

Per-core algorithm (flash-attention style, S^T orientation), v2:
  - Stage Q, K as fp16 via SWDGE cast DMA into [128 seq, (h d)] tiles,
    then DMA XBAR-transpose (16x128 tiles, sync queue) into qT/kT
    [128=(h,d), 4096].  The PE does no staging work at all.
  - V per head into vplus [128, 32*65] fp16: 64 V columns plus a ones
    column per 128-row k-block (fused softmax denominator).
  - Main loop over (G, j): one 128-wide k-block j per iteration, both
    heads:
      mm1: S^T[kj, qi] for h0/h1 emitted back-to-back into one combined
           PSUM tile [128, 1024] with tile_position=(h*64, 0) so the two
           K=64 matmuls stream concurrently on disjoint PE row groups.
      exp: split across three engines.  ACT computes exact
           exp(s*0.125) -> fp16.  DVE / GPSIMD compute a Schraudolph
           approximation: t = (s + B/A)*A truncated to int16 and
           bitcast as fp16 equals 2^(s*0.125*log2 e) up to a constant
           factor (cancels in softmax) and a +-2% sawtooth.  For
           diagonal blocks the multiplier A is a precomputed per-element
           tensor (A where causally valid, 0 where masked) so masked
           lanes produce exactly +0.0.  G0 runs on the exact ACT path
           with 0/1 mask multiplies (small-denominator safety).
      mm2: O[qi, 64+1] += expS^T_chunk.T @ vplus_j, deferred two
           iterations (software pipelining keeps the PE queue full so
           the PE p-state can ramp to 2.4 GHz).
  - Normalize: batched reciprocal of the ones-columns (DVE), row-scale
    on GPSIMD, DMA out on the sync queue.
"""

import math

import numpy as np

SEQ = 4096
NHEAD = 16
HDIM = 64
NCORES = 8
HPC = NHEAD // NCORES  # heads per core = 2
SCALE = 0.125

# Schraudolph exp2 constants for fp16 bitcast output.
# t = (s + B/A) * A ; P = bitcast_fp16(int16(t)) ~= C * exp(s * SCALE)
EXP_A = 1024.0 / math.log(2.0) * SCALE  # 184.665
EXP_CORR = -0.0434  # sawtooth centering (constant factor cancels)
EXP_B = 15360.0 + EXP_CORR * 1024.0 + 0.5  # +0.5 centers the truncation
EXP_BOA = EXP_B / EXP_A

_NC_CACHE = {}
LAST_RESULT = {}


def build_attention_nc(seq=SEQ, hpc=HPC, hdim=HDIM, gp_exp=True, split_waits=True):
    """Build the SPMD Bass program for one core handling `hpc` heads."""
    import concourse.bass as bass
    import concourse.mybir as mybir
    import concourse.tile as tile

    f32 = mybir.dt.float32
    fp16 = mybir.dt.float16
    i16 = mybir.dt.int16
    Exp = mybir.ActivationFunctionType.Exp

    assert hpc == 2 and hdim == 64, "layout hardcoded for 2 heads x 64 dim"
    assert seq % 512 == 0
    nt = seq // 128   # number of 128-row seq tiles (32)
    ng = seq // 512   # number of 512-wide q groups (8)

    nc = bass.Bass()
    q = nc.dram_tensor("q", [seq, hpc, hdim], f32, kind="ExternalInput").ap()
    k = nc.dram_tensor("k", [seq, hpc, hdim], f32, kind="ExternalInput").ap()
    v = nc.dram_tensor("v", [seq, hpc, hdim], f32, kind="ExternalOutput" if False else "ExternalInput").ap()
    o = nc.dram_tensor("o", [seq, hpc * hdim], f32, kind="ExternalOutput").ap()

    with tile.TileContext(nc) as tc:
        with (
            tc.tile_pool(name="persist", bufs=1) as persist,
            tc.tile_pool(name="ldstage", bufs=4) as ld_pool,
            tc.tile_pool(name="pexp", bufs=4) as pexp_pool,
            tc.tile_pool(name="outp", bufs=6) as out_pool,
            tc.tile_pool(name="small", bufs=8) as small_pool,
        ):
            # ---- persistent SBUF tensors ----------------------------------
            qT = persist.tile([128, seq], fp16, tag="qT")
            kT = persist.tile([128, seq], fp16, tag="kT")
            vplus = [
                persist.tile([128, nt * (hdim + 1)], fp16, tag=f"vplus{h}", name=f"vplus{h}")
                for h in range(hpc)
            ]
            # amask_t[kj, qi] = EXP_A where kj + 128*t <= qi else 0.0
            # (fused causal mask for the Schraudolph path)
            amask = [persist.tile([128, 512], f32, tag=f"amask{t}", name=f"amask{t}") for t in range(4)]
            # mask01_t: 1/0 causal masks, fp16, for the G0 exact path.
            mask01 = [persist.tile([128, 512], fp16, tag=f"mask01_{t}", name=f"mask01_{t}") for t in range(4)]

            def build_mask01(t):
                nc.vector.memset(mask01[t], 1.0)
                nc.gpsimd.affine_select(
                    out=mask01[t][:],
                    in_=mask01[t][:],
                    compare_op=mybir.AluOpType.is_ge,
                    fill=0.0,
                    base=-128 * t,
                    pattern=[[1, 512]],
                    channel_multiplier=-1,
                )

            def build_amasks():
                for t in range(4):
                    nc.vector.memset(amask[t], EXP_A)
                    nc.gpsimd.affine_select(
                        out=amask[t][:],
                        in_=amask[t][:],
                        compare_op=mybir.AluOpType.is_ge,
                        fill=0.0,
                        base=-128 * t,
                        pattern=[[1, 512]],
                        channel_multiplier=-1,
                    )

            def load_v_chunk(c, tiles_per_chunk):
                # v chunk c covers k-tiles [c*tpc, (c+1)*tpc)
                t0 = c * tiles_per_chunk
                t1 = min(nt, t0 + tiles_per_chunk)
                for h in range(hpc):
                    nc.gpsimd.dma_start(
                        out=vplus[h]
                        .rearrange("p (t x) -> p t x", x=hdim + 1)[:, t0:t1, 0:hdim],
                        in_=v[t0 * 128 : t1 * 128, h, :].rearrange(
                            "(t p) d -> p t d", p=128
                        ),
                    )

            # ---- staging: SWDGE cast-load + PE transpose ------------------
            # Super-chunks of 8 k-tiles. The PE transposes each staged
            # [128 seq, 128 (h d)] tile into a PSUM buffer borrowed from the
            # mm1 score pool (bitcast fp16), then one wide ACT/DVE copy moves
            # 8 transposed tiles into qT/kT. Staging for super-chunk c+1 is
            # emitted AFTER main-loop groups G=2c,2c+1 so the PE pipeline
            # never serializes behind the whole staging phase.
            schunk = 8
            nsc = nt // schunk  # 4 super-chunks
            identity = persist.tile([128, 128], fp16, tag="identity")
            from concourse.masks import make_identity

            make_identity(nc, identity[:])

            # memset the ones columns of vplus before any v data lands
            for h in range(hpc):
                nc.vector.memset(vplus[h], 1.0)

            with (
                tc.tile_pool(name="psum_s", bufs=3, space="PSUM") as psum_s_pool,
                tc.tile_pool(name="psum_o", bufs=1, space="PSUM") as psum_o_pool,
            ):
                copy_rot = [0]

                def stage_tiles(t0, ntile):
                    for src_t, dstT in ((k, kT), (q, qT)):
                        src_r = src_t.rearrange("(t p) h d -> p t (h d)", p=128)
                        st = ld_pool.tile([128, schunk * 128], fp16, tag="ldstage")
                        nc.gpsimd.dma_start(
                            out=st.rearrange("p (t x) -> p t x", x=128)[:, 0:ntile, :],
                            in_=src_r[:, t0 : t0 + ntile, :],
                        )
                        tr = psum_s_pool.tile([128, 1024], f32, tag="ps", name="ps")
                        trv = tr.bitcast(fp16)  # [128, 2048] fp16 view
                        for tt in range(ntile):
                            nc.tensor.transpose(
                                trv[:, tt * 128 : (tt + 1) * 128],
                                st[:, tt * 128 : (tt + 1) * 128],
                                identity[:],
                            )
                        # one wide PSUM->SBUF copy per chunk, alternating
                        dst = dstT[:, t0 * 128 : (t0 + ntile) * 128]
                        if copy_rot[0] % 2 == 0:
                            nc.scalar.copy(dst, trv[:, 0 : ntile * 128])
                        else:
                            nc.vector.tensor_copy(dst, trv[:, 0 : ntile * 128])
                        copy_rot[0] += 1

                def stage_superchunk(c):
                    if c == 0:
                        stage_tiles(0, 4)
                        for t in range(4):
                            build_mask01(t)
                        load_v_tiles(0, 4)
                        stage_tiles(4, 4)
                        load_v_tiles(4, 8)
                        build_amasks()
                    else:
                        stage_tiles(c * schunk, schunk)
                        load_v_tiles(c * schunk, (c + 1) * schunk)

                def load_v_tiles(t0, t1):
                    for h in range(hpc):
                        nc.gpsimd.dma_start(
                            out=vplus[h]
                            .rearrange("p (t x) -> p t x", x=hdim + 1)[:, t0:t1, 0:hdim],
                            in_=v[t0 * 128 : t1 * 128, h, :].rearrange(
                                "(t p) d -> p t d", p=128
                            ),
                        )

                loop = _MainLoop(
                    nc, mybir, ng, hdim, psum_s_pool, psum_o_pool, pexp_pool,
                    out_pool, small_pool, qT, kT, vplus, amask, mask01, o,
                    hpc, Exp,
                )
                for c in range(nsc):
                    stage_superchunk(c)
                    loop.emit_group(2 * c)
                    loop.emit_group(2 * c + 1)
                loop.flush()
    if split_waits:
        _split_multi_waits(nc)
    return nc


def _split_multi_waits(nc):
    """Walrus's codegen accepts at most one sync-wait per instruction on
    this toolchain. Hoist extra waits into standalone single-wait NoOps on
    the same engine queue (same semantics: the sequencer stalls in order)."""
    import concourse.mybir as mybir

    nsplit = 0
    for blk in nc.m.functions[0].blocks:
        newl = []
        for ins in blk.instructions:
            si = getattr(ins, "sync_info", None)
            if si is not None and si.on_wait and len(si.on_wait) > 1:
                waits = list(si.on_wait)
                for w in waits[:-1]:
                    newl.append(
                        mybir.InstNoOp(
                            name=f"{ins.name}-wsplit{nsplit}",
                            sync_info=mybir.SyncInfo(on_wait=[w], on_update=[]),
                            bass_nofuse=True,
                            engine=ins.engine,
                            ins=[],
                            outs=[],
                        )
                    )
                    nsplit += 1
                ins.sync_info = mybir.SyncInfo(
                    on_wait=[waits[-1]], on_update=list(si.on_update or [])
                )
            newl.append(ins)
        blk.instructions = newl
    return nsplit


class _MainLoop:
    """Emits main-loop groups interleaved with staging.

    One iteration = one 128-wide k-block j for both heads.  mm2 for
    iteration g is deferred until after mm1 of iteration g+2 (the PE
    queue always holds independent work while ACT/DVE compute exp).
    """

    def __init__(self, nc, mybir, ng, hdim, psum_s_pool, psum_o_pool,
                 pexp_pool, out_pool, small_pool, qT, kT, vplus, amask,
                 mask01, o, hpc, Exp):
        self.nc = nc
        self.mybir = mybir
        self.ng = ng
        self.hdim = hdim
        self.psum_s_pool = psum_s_pool
        self.psum_o_pool = psum_o_pool
        self.pexp_pool = pexp_pool
        self.out_pool = out_pool
        self.small_pool = small_pool
        self.qT = qT
        self.kT = kT
        self.vplus = vplus
        self.amask = amask
        self.mask01 = mask01
        self.o = o
        self.hpc = hpc
        self.Exp = Exp
        self.f32 = mybir.dt.float32
        self.fp16 = mybir.dt.float16
        self.i16 = mybir.dt.int16
        self.add = mybir.AluOpType.add
        self.mult = mybir.AluOpType.mult
        self.Copy = mybir.ActivationFunctionType.Copy
        self.pending = []
        # greedy ACT/DVE balance counters (estimated busy ns)
        self.busy = {"act": 0.0, "dve": 0.0}
        self.ACT_LAT = 300.0
        self.DVE_LAT = 145.0

    # --- exp paths -----------------------------------------------------
    def exp_act_pair(self, pe, ps):
        nc = self.nc
        nc.scalar.activation(out=pe[:, 0:1024], in_=ps[:, 0:1024],
                             func=self.Exp, scale=SCALE)
        self.busy["act"] += 1024 * 0.833 + self.ACT_LAT

    def exp_dve_pair(self, pe, ps, t, q0):
        nc = self.nc
        if t >= 0:
            # triangle columns [q0, q0+128) -> DVE Schraudolph with fused
            # mask; fully-valid columns [q0+128, 512) -> exact ACT exp.
            q1 = q0 + 128
            nc.vector.scalar_tensor_tensor(
                out=pe.rearrange("p (h x) -> p h x", h=2)[:, :, q0:q1].bitcast(self.i16),
                in0=ps.rearrange("p (h x) -> p h x", h=2)[:, :, q0:q1],
                scalar=EXP_BOA,
                in1=self.amask[t][:, q0:q1].rearrange("p x -> p () x").broadcast_to([128, 2, 128]),
                op0=self.add,
                op1=self.mult,
            )
            self.busy["dve"] += 2 * 128 * 1.042 + self.DVE_LAT
            if q1 < 512:
                w = 512 - q1
                nc.scalar.activation(
                    out=pe.rearrange("p (h x) -> p h x", h=2)[:, :, q1:512],
                    in_=ps.rearrange("p (h x) -> p h x", h=2)[:, :, q1:512],
                    func=self.Exp,
                    scale=SCALE,
                )
                self.busy["act"] += 2 * w * 0.833 + self.ACT_LAT
        else:
            nc.vector.tensor_scalar(
                out=pe[:, 0:1024].bitcast(self.i16),
                in0=ps[:, 0:1024],
                scalar1=EXP_BOA,
                scalar2=EXP_A,
                op0=self.add,
                op1=self.mult,
            )
            self.busy["dve"] += 1024 * 1.042 + self.DVE_LAT

    # --- mm2 + finals --------------------------------------------------
    def emit_mm2(self, st):
        nc = self.nc
        G, j, po, pe, njs, last = st
        t = j - 4 * G
        hdim = self.hdim
        for h in range(self.hpc):
            for c in range(4):
                if t > c:
                    continue  # chunk fully masked -> zero contribution
                nc.tensor.matmul(
                    po[h][:, c * 128 : c * 128 + hdim + 1],
                    lhsT=pe[:, h * 512 + c * 128 : h * 512 + (c + 1) * 128],
                    rhs=self.vplus[h][:, j * 65 : j * 65 + hdim + 1],
                    start=(j == 0 and c == 0),
                    stop=(j == njs - 1 and c == 3),
                    skip_group_check=True,
                )

    def emit_finals(self, G, po):
        # Copy po out of PSUM immediately (frees the bank for the next G's
        # mm2 accumulation), then do reciprocal+normalize from SBUF so the
        # normalize can run on the otherwise-idle GPSIMD engine.
        nc = self.nc
        hdim = self.hdim
        pos = []
        for h in range(self.hpc):
            p_sb = self.out_pool.tile([128, 260], self.f32, tag="posb", name="posb")
            src_ap = po[h].rearrange("p (c x) -> p c x", x=128)[:, :, 0 : hdim + 1]
            dst_ap = p_sb.rearrange("p (c x) -> p c x", x=hdim + 1)
            if self.busy["act"] <= self.busy["dve"]:
                nc.scalar.copy(dst_ap, src_ap)
                self.busy["act"] += 260 * 0.833 + self.ACT_LAT
            else:
                nc.vector.tensor_copy(dst_ap, src_ap)
                self.busy["dve"] += 260 * 1.042 + self.DVE_LAT
            pos.append(p_sb)
        recs = []
        for h in range(self.hpc):
            rec4 = self.small_pool.tile([128, 4], self.f32, tag="rec4", name="rec4")
            nc.vector.reciprocal(
                rec4,
                pos[h].rearrange("p (c x) -> p c x", x=hdim + 1)[:, :, hdim : hdim + 1],
            )
            recs.append(rec4)
        ob = self.out_pool.tile([128, 4 * self.hpc * hdim], self.f32, tag="ob", name="ob")
        obv = ob.rearrange("p (c h d) -> p c h d", c=4, h=self.hpc)
        for h in range(self.hpc):
            # one batched normalize per head: broadcast rec4 over the 64
            # feature columns with a 0-stride AP
            rec_b = recs[h].broadcast_to([128, 4, hdim])
            nc.gpsimd.tensor_mul(
                obv[:, :, h, :],
                pos[h].rearrange("p (c x) -> p c x", x=hdim + 1)[:, :, 0:hdim],
                rec_b,
            )
        nc.sync.dma_start(
            out=self.o[G * 512 : (G + 1) * 512, :].rearrange("(c p) f -> p c f", p=128),
            in_=ob.rearrange("p (c f) -> p c f", c=4),
        )

    # --- per-group emission --------------------------------------------
    def emit_group(self, G):
        nc = self.nc
        njs = 4 * G + 4
        po = [
            self.psum_o_pool.tile([128, 512], self.f32, tag=f"po{h}", name=f"po{h}")
            for h in range(self.hpc)
        ]
        for j in range(njs):
            t = j - 4 * G
            ps = self.psum_s_pool.tile([128, 1024], self.f32, tag="ps", name="ps")
            q0 = 128 * t if (t > 0 and G >= 1) else 0
            for h in range(self.hpc):
                nc.tensor.matmul(
                    ps[:, h * 512 + q0 : (h + 1) * 512],
                    lhsT=self.kT[h * 64 : (h + 1) * 64, j * 128 : (j + 1) * 128],
                    rhs=self.qT[h * 64 : (h + 1) * 64, G * 512 + q0 : (G + 1) * 512],
                    start=True,
                    stop=True,
                    tile_position=(h * 64, 0),
                )
            pe = self.pexp_pool.tile([128, 1024], self.fp16, tag="pexp", name="pexp")
            if G == 0:
                # exact path with 0/1 mask multiplies (DVE, fp16 2x)
                self.exp_act_pair(pe, ps)
                for h in range(self.hpc):
                    nc.vector.tensor_mul(
                        pe[:, h * 512 : (h + 1) * 512],
                        pe[:, h * 512 : (h + 1) * 512],
                        self.mask01[t][:],
                    )
                    self.busy["dve"] += 512 * 0.521 + self.DVE_LAT
            elif t >= 0:
                self.exp_dve_pair(pe, ps, t, q0)
            elif self.busy["act"] <= self.busy["dve"]:
                self.exp_act_pair(pe, ps)
            else:
                self.exp_dve_pair(pe, ps, t, 0)
            self.pending.append((G, j, po, pe, njs, j == njs - 1))
            if len(self.pending) > 3:
                st = self.pending.pop(0)
                self.emit_mm2(st)
                if st[5]:
                    self.emit_finals(st[0], st[2])

    def flush(self):
        for st in self.pending:
            self.emit_mm2(st)
            if st[5]:
                self.emit_finals(st[0], st[2])
        self.pending = []


def _ensure_ntff_hook():
    """The image's antenv package lacks axon_hooks; provide it so
    run_bass_kernel_spmd's trace path works (or degrades gracefully)."""
    import sys
    import types

    try:
        import antenv.axon_hooks  # noqa: F401

        return
    except ImportError:
        pass
    mod = types.ModuleType("antenv.axon_hooks")
    state = {"hook": None}
    mod.set_axon_ntff_profile_hook = lambda h: state.__setitem__("hook", h)
    mod.get_axon_ntff_profile_hook = lambda: state["hook"]
    try:
        from trn_agent_boot.trn_boot import _ntff_profile_via_ctypes

        state["hook"] = _ntff_profile_via_ctypes("/opt/axon/libaxon_pjrt.so")
    except Exception:
        state["hook"] = None
    sys.modules["antenv.axon_hooks"] = mod


def kernel(q, k, v):
    """Full-input entry point: q, k, v [4096, 16, 64] fp32 -> [4096, 1024]."""
    import sys

    if "/opt/trn_rl_repo" not in sys.path:
        sys.path.insert(0, "/opt/trn_rl_repo")
    _ensure_ntff_hook()
    from concourse.bass_utils import run_bass_kernel_spmd

    q = np.asarray(q, dtype=np.float32)
    k = np.asarray(k, dtype=np.float32)
    v = np.asarray(v, dtype=np.float32)
    seq, nhead, hdim = q.shape

    if "nc" not in _NC_CACHE:
        _NC_CACHE["nc"] = build_attention_nc(seq=seq, hpc=HPC, hdim=hdim)
    nc = _NC_CACHE["nc"]

    in_maps = []
    for c in range(NCORES):
        hs = slice(c * HPC, (c + 1) * HPC)
        in_maps.append(
            {
                "q": np.ascontiguousarray(q[:, hs, :]),
                "k": np.ascontiguousarray(k[:, hs, :]),
                "v": np.ascontiguousarray(v[:, hs, :]),
            }
        )
    res = run_bass_kernel_spmd(nc, in_maps, core_ids=list(range(NCORES)))
    LAST_RESULT["exec_time_ns"] = res.exec_time_ns
    try:
        iat = res.instructions_and_trace
        LAST_RESULT["trace_path"] = iat[1] if iat else None
    except Exception:
        LAST_RESULT["trace_path"] = None
    outs = [res.results[c]["o"] for c in range(NCORES)]
    return np.concatenate(outs, axis=1)


# revision 20
# speedup vs baseline: 2.3050x; 1.0021x over previous
"""Trainium2 Bass kernel for multi-head causal attention.

Problem: q, k, v of shape [4096, 16, 64] (seq, heads, head_dim) fp32.
  out = softmax(causal(q @ k^T / 8)) @ v, reshaped to [4096, 1024].

Sharding: heads are split across 8 NeuronCores (2 heads per core).
Each core runs the same SPMD Bass program on its own 2 heads; the host
concatenates the per-core [4096, 128] outputs along the feature dim.

Per-core algorithm (flash-attention style, S^T orientation), v2:
  - Stage Q, K as fp16 via SWDGE cast DMA into [128 seq, (h d)] tiles,
    then DMA XBAR-transpose (16x128 tiles, sync queue) into qT/kT
    [128=(h,d), 4096].  The PE does no staging work at all.
  - V per head into vplus [128, 32*65] fp16: 64 V columns plus a ones
    column per 128-row k-block (fused softmax denominator).
  - Main loop over (G, j): one 128-wide k-block j per iteration, both
    heads:
      mm1: S^T[kj, qi] for h0/h1 emitted back-to-back into one combined
           PSUM tile [128, 1024] with tile_position=(h*64, 0) so the two
           K=64 matmuls stream concurrently on disjoint PE row groups.
      exp: split across three engines.  ACT computes exact
           exp(s*0.125) -> fp16.  DVE / GPSIMD compute a Schraudolph
           approximation: t = (s + B/A)*A truncated to int16 and
           bitcast as fp16 equals 2^(s*0.125*log2 e) up to a constant
           factor (cancels in softmax) and a +-2% sawtooth.  For
           diagonal blocks the multiplier A is a precomputed per-element
           tensor (A where causally valid, 0 where masked) so masked
           lanes produce exactly +0.0.  G0 runs on the exact ACT path
           with 0/1 mask multiplies (small-denominator safety).
      mm2: O[qi, 64+1] += expS^T_chunk.T @ vplus_j, deferred two
           iterations (software pipelining keeps the PE queue full so
           the PE p-state can ramp to 2.4 GHz).
  - Normalize: batched reciprocal of the ones-columns (DVE), row-scale
    on GPSIMD, DMA out on the sync queue.
"""

import math

import numpy as np

SEQ = 4096
NHEAD = 16
HDIM = 64
NCORES = 8
HPC = NHEAD // NCORES  # heads per core = 2
SCALE = 0.125

# Schraudolph exp2 constants for fp16 bitcast output.
# t = (s + B/A) * A ; P = bitcast_fp16(int16(t)) ~= C * exp(s * SCALE)
EXP_A = 1024.0 / math.log(2.0) * SCALE  # 184.665
EXP_CORR = -0.0434  # sawtooth centering (constant factor cancels)
EXP_B = 15360.0 + EXP_CORR * 1024.0 + 0.5  # +0.5 centers the truncation
EXP_BOA = EXP_B / EXP_A

_NC_CACHE = {}
LAST_RESULT = {}


def build_attention_nc(seq=SEQ, hpc=HPC, hdim=HDIM, gp_exp=True, split_waits=True):
    """Build the SPMD Bass program for one core handling `hpc` heads."""
    import concourse.bass as bass
    import concourse.mybir as mybir
    import concourse.tile as tile

    f32 = mybir.dt.float32
    fp16 = mybir.dt.float16
    i16 = mybir.dt.int16
    Exp = mybir.ActivationFunctionType.Exp

    assert hpc == 2 and hdim == 64, "layout hardcoded for 2 heads x 64 dim"
    assert seq % 512 == 0
    nt = seq // 128   # number of 128-row seq tiles (32)
    ng = seq // 512   # number of 512-wide q groups (8)

    nc = bass.Bass()
    q = nc.dram_tensor("q", [seq, hpc, hdim], f32, kind="ExternalInput").ap()
    k = nc.dram_tensor("k", [seq, hpc, hdim], f32, kind="ExternalInput").ap()
    v = nc.dram_tensor("v", [seq, hpc, hdim], f32, kind="ExternalOutput" if False else "ExternalInput").ap()
    o = nc.dram_tensor("o", [seq, hpc * hdim], f32, kind="ExternalOutput").ap()

    with tile.TileContext(nc) as tc:
        with (
            tc.tile_pool(name="persist", bufs=1) as persist,
            tc.tile_pool(name="ldstage", bufs=4) as ld_pool,
            tc.tile_pool(name="pexp", bufs=4) as pexp_pool,
            tc.tile_pool(name="outp", bufs=6) as out_pool,
            tc.tile_pool(name="small", bufs=8) as small_pool,
        ):
            # ---- persistent SBUF tensors ----------------------------------
            qT = persist.tile([128, seq], fp16, tag="qT")
            kT = persist.tile([128, seq], fp16, tag="kT")
            vplus = [
                persist.tile([128, nt * (hdim + 1)], fp16, tag=f"vplus{h}", name=f"vplus{h}")
                for h in range(hpc)
            ]
            # amask_t[kj, qi] = EXP_A where kj + 128*t <= qi else 0.0
            # (fused causal mask for the Schraudolph path)
            amask = [persist.tile([128, 512], f32, tag=f"amask{t}", name=f"amask{t}") for t in range(4)]
            # mask01_t: 1/0 causal masks, fp16, for the G0 exact path.
            mask01 = [persist.tile([128, 512], fp16, tag=f"mask01_{t}", name=f"mask01_{t}") for t in range(4)]

            def build_mask01(t):
                nc.vector.memset(mask01[t], 1.0)
                nc.gpsimd.affine_select(
                    out=mask01[t][:],
                    in_=mask01[t][:],
                    compare_op=mybir.AluOpType.is_ge,
                    fill=0.0,
                    base=-128 * t,
                    pattern=[[1, 512]],
                    channel_multiplier=-1,
                )

            def build_amasks():
                # amask = EXP_A * mask01 (DVE, cast fp16 -> fp32)
                for t in range(4):
                    nc.vector.tensor_scalar(
                        out=amask[t][:],
                        in0=mask01[t][:],
                        scalar1=float(EXP_A),
                        scalar2=None,
                        op0=mybir.AluOpType.mult,
                    )

            def load_v_chunk(c, tiles_per_chunk):
                # v chunk c covers k-tiles [c*tpc, (c+1)*tpc)
                t0 = c * tiles_per_chunk
                t1 = min(nt, t0 + tiles_per_chunk)
                for h in range(hpc):
                    nc.gpsimd.dma_start(
                        out=vplus[h]
                        .rearrange("p (t x) -> p t x", x=hdim + 1)[:, t0:t1, 0:hdim],
                        in_=v[t0 * 128 : t1 * 128, h, :].rearrange(
                            "(t p) d -> p t d", p=128
                        ),
                    )

            # ---- staging: SWDGE cast-load + PE transpose ------------------
            # Super-chunks of 8 k-tiles. The PE transposes each staged
            # [128 seq, 128 (h d)] tile into a PSUM buffer borrowed from the
            # mm1 score pool (bitcast fp16), then one wide ACT/DVE copy moves
            # 8 transposed tiles into qT/kT. Staging for super-chunk c+1 is
            # emitted AFTER main-loop groups G=2c,2c+1 so the PE pipeline
            # never serializes behind the whole staging phase.
            schunk = 8
            nsc = nt // schunk  # 4 super-chunks
            identity = persist.tile([128, 128], fp16, tag="identity")
            from concourse.masks import make_identity

            make_identity(nc, identity[:])

            # memset the ones columns of vplus before any v data lands
            for h in range(hpc):
                nc.vector.memset(vplus[h], 1.0)

            with (
                tc.tile_pool(name="psum_s", bufs=3, space="PSUM") as psum_s_pool,
                tc.tile_pool(name="psum_o", bufs=1, space="PSUM") as psum_o_pool,
            ):
                copy_rot = [0]

                def stage_tiles(t0, ntile):
                    for src_t, dstT in ((k, kT), (q, qT)):
                        src_r = src_t.rearrange("(t p) h d -> p t (h d)", p=128)
                        st = ld_pool.tile([128, schunk * 128], fp16, tag="ldstage")
                        nc.gpsimd.dma_start(
                            out=st.rearrange("p (t x) -> p t x", x=128)[:, 0:ntile, :],
                            in_=src_r[:, t0 : t0 + ntile, :],
                        )
                        tr = psum_s_pool.tile([128, 1024], f32, tag="ps", name="ps")
                        trv = tr.bitcast(fp16)  # [128, 2048] fp16 view
                        for tt in range(ntile):
                            nc.tensor.transpose(
                                trv[:, tt * 128 : (tt + 1) * 128],
                                st[:, tt * 128 : (tt + 1) * 128],
                                identity[:],
                            )
                        # one wide PSUM->SBUF copy per chunk, alternating
                        dst = dstT[:, t0 * 128 : (t0 + ntile) * 128]
                        if copy_rot[0] % 2 == 0:
                            nc.scalar.copy(dst, trv[:, 0 : ntile * 128])
                        else:
                            nc.vector.tensor_copy(dst, trv[:, 0 : ntile * 128])
                        copy_rot[0] += 1

                def stage_superchunk(c):
                    stage_tiles(c * schunk, schunk)
                    load_v_tiles(c * schunk, (c + 1) * schunk)

                def load_v_tiles(t0, t1):
                    for h in range(hpc):
                        nc.gpsimd.dma_start(
                            out=vplus[h]
                            .rearrange("p (t x) -> p t x", x=hdim + 1)[:, t0:t1, 0:hdim],
                            in_=v[t0 * 128 : t1 * 128, h, :].rearrange(
                                "(t p) d -> p t d", p=128
                            ),
                        )

                loop = _MainLoop(
                    nc, mybir, ng, hdim, psum_s_pool, psum_o_pool, pexp_pool,
                    out_pool, small_pool, qT, kT, vplus, amask, mask01, o,
                    hpc, Exp,
                )
                # lead-in: G0 only needs k/q tiles 0-3 and v tiles 0-3;
                # emit it between the two halves of super-chunk 0 so the
                # in-order PE queue reaches G0's matmuls immediately.
                stage_tiles(0, 4)
                for t in range(4):
                    build_mask01(t)
                load_v_tiles(0, 4)
                loop.emit_group(0)
                stage_tiles(4, 4)
                load_v_tiles(4, 8)
                build_amasks()
                loop.emit_group(1)
                for c in range(1, nsc):
                    stage_superchunk(c)
                    loop.emit_group(2 * c)
                    loop.emit_group(2 * c + 1)
                loop.flush()
    if split_waits:
        _split_multi_waits(nc)
    return nc


def _split_multi_waits(nc):
    """Walrus's codegen accepts at most one sync-wait per instruction on
    this toolchain. Hoist extra waits into standalone single-wait NoOps on
    the same engine queue (same semantics: the sequencer stalls in order)."""
    import concourse.mybir as mybir

    nsplit = 0
    for blk in nc.m.functions[0].blocks:
        newl = []
        for ins in blk.instructions:
            si = getattr(ins, "sync_info", None)
            if si is not None and si.on_wait and len(si.on_wait) > 1:
                waits = list(si.on_wait)
                for w in waits[:-1]:
                    newl.append(
                        mybir.InstNoOp(
                            name=f"{ins.name}-wsplit{nsplit}",
                            sync_info=mybir.SyncInfo(on_wait=[w], on_update=[]),
                            bass_nofuse=True,
                            engine=ins.engine,
                            ins=[],
                            outs=[],
                        )
                    )
                    nsplit += 1
                ins.sync_info = mybir.SyncInfo(
                    on_wait=[waits[-1]], on_update=list(si.on_update or [])
                )
            newl.append(ins)
        blk.instructions = newl
    return nsplit


class _MainLoop:
    """Emits main-loop groups interleaved with staging.

    One iteration = one 128-wide k-block j for both heads.  mm2 for
    iteration g is deferred until after mm1 of iteration g+2 (the PE
    queue always holds independent work while ACT/DVE compute exp).
    """

    def __init__(self, nc, mybir, ng, hdim, psum_s_pool, psum_o_pool,
                 pexp_pool, out_pool, small_pool, qT, kT, vplus, amask,
                 mask01, o, hpc, Exp):
        self.nc = nc
        self.mybir = mybir
        self.ng = ng
        self.hdim = hdim
        self.psum_s_pool = psum_s_pool
        self.psum_o_pool = psum_o_pool
        self.pexp_pool = pexp_pool
        self.out_pool = out_pool
        self.small_pool = small_pool
        self.qT = qT
        self.kT = kT
        self.vplus = vplus
        self.amask = amask
        self.mask01 = mask01
        self.o = o
        self.hpc = hpc
        self.Exp = Exp
        self.f32 = mybir.dt.float32
        self.fp16 = mybir.dt.float16
        self.i16 = mybir.dt.int16
        self.add = mybir.AluOpType.add
        self.mult = mybir.AluOpType.mult
        self.Copy = mybir.ActivationFunctionType.Copy
        self.pending = []
        # greedy ACT/DVE balance counters (estimated busy ns)
        self.busy = {"act": 0.0, "dve": 0.0}
        self.ACT_LAT = 300.0
        self.DVE_LAT = 145.0

    # --- exp paths -----------------------------------------------------
    def exp_act_pair(self, pe, ps):
        nc = self.nc
        nc.scalar.activation(out=pe[:, 0:1024], in_=ps[:, 0:1024],
                             func=self.Exp, scale=SCALE)
        self.busy["act"] += 1024 * 0.833 + self.ACT_LAT

    def exp_dve_pair(self, pe, ps, t, q0):
        nc = self.nc
        if t >= 0:
            # triangle columns [q0, q0+128) -> DVE Schraudolph with fused
            # mask; fully-valid columns [q0+128, 512) -> exact ACT exp.
            q1 = q0 + 128
            nc.vector.scalar_tensor_tensor(
                out=pe.rearrange("p (h x) -> p h x", h=2)[:, :, q0:q1].bitcast(self.i16),
                in0=ps.rearrange("p (h x) -> p h x", h=2)[:, :, q0:q1],
                scalar=EXP_BOA,
                in1=self.amask[t][:, q0:q1].rearrange("p x -> p () x").broadcast_to([128, 2, 128]),
                op0=self.add,
                op1=self.mult,
            )
            self.busy["dve"] += 2 * 128 * 1.042 + self.DVE_LAT
            if q1 < 512:
                w = 512 - q1
                nc.scalar.activation(
                    out=pe.rearrange("p (h x) -> p h x", h=2)[:, :, q1:512],
                    in_=ps.rearrange("p (h x) -> p h x", h=2)[:, :, q1:512],
                    func=self.Exp,
                    scale=SCALE,
                )
                self.busy["act"] += 2 * w * 0.833 + self.ACT_LAT
        else:
            nc.vector.tensor_scalar(
                out=pe[:, 0:1024].bitcast(self.i16),
                in0=ps[:, 0:1024],
                scalar1=EXP_BOA,
                scalar2=EXP_A,
                op0=self.add,
                op1=self.mult,
            )
            self.busy["dve"] += 1024 * 1.042 + self.DVE_LAT

    # --- mm2 + finals --------------------------------------------------
    def emit_mm2(self, st):
        nc = self.nc
        G, j, po, pe, njs, last = st
        t = j - 4 * G
        hdim = self.hdim
        for h in range(self.hpc):
            for c in range(4):
                if t > c:
                    continue  # chunk fully masked -> zero contribution
                nc.tensor.matmul(
                    po[h][:, c * 128 : c * 128 + hdim + 1],
                    lhsT=pe[:, h * 512 + c * 128 : h * 512 + (c + 1) * 128],
                    rhs=self.vplus[h][:, j * 65 : j * 65 + hdim + 1],
                    start=(j == 0 and c == 0),
                    stop=(j == njs - 1 and c == 3),
                    skip_group_check=True,
                )

    def emit_finals(self, G, po):
        # Copy po out of PSUM immediately (frees the bank for the next G's
        # mm2 accumulation), then do reciprocal+normalize from SBUF so the
        # normalize can run on the otherwise-idle GPSIMD engine.
        nc = self.nc
        hdim = self.hdim
        pos = []
        for h in range(self.hpc):
            p_sb = self.out_pool.tile([128, 260], self.f32, tag="posb", name="posb")
            src_ap = po[h].rearrange("p (c x) -> p c x", x=128)[:, :, 0 : hdim + 1]
            dst_ap = p_sb.rearrange("p (c x) -> p c x", x=hdim + 1)
            if self.busy["act"] <= self.busy["dve"]:
                nc.scalar.copy(dst_ap, src_ap)
                self.busy["act"] += 260 * 0.833 + self.ACT_LAT
            else:
                nc.vector.tensor_copy(dst_ap, src_ap)
                self.busy["dve"] += 260 * 1.042 + self.DVE_LAT
            pos.append(p_sb)
        recs = []
        for h in range(self.hpc):
            rec4 = self.small_pool.tile([128, 4], self.f32, tag="rec4", name="rec4")
            nc.vector.reciprocal(
                rec4,
                pos[h].rearrange("p (c x) -> p c x", x=hdim + 1)[:, :, hdim : hdim + 1],
            )
            recs.append(rec4)
        ob = self.out_pool.tile([128, 4 * self.hpc * hdim], self.f32, tag="ob", name="ob")
        obv = ob.rearrange("p (c h d) -> p c h d", c=4, h=self.hpc)
        for h in range(self.hpc):
            # one batched normalize per head: broadcast rec4 over the 64
            # feature columns with a 0-stride AP
            rec_b = recs[h].broadcast_to([128, 4, hdim])
            nc.gpsimd.tensor_mul(
                obv[:, :, h, :],
                pos[h].rearrange("p (c x) -> p c x", x=hdim + 1)[:, :, 0:hdim],
                rec_b,
            )
        nc.sync.dma_start(
            out=self.o[G * 512 : (G + 1) * 512, :].rearrange("(c p) f -> p c f", p=128),
            in_=ob.rearrange("p (c f) -> p c f", c=4),
        )

    # --- per-group emission --------------------------------------------
    def emit_group(self, G):
        nc = self.nc
        njs = 4 * G + 4
        po = [
            self.psum_o_pool.tile([128, 512], self.f32, tag=f"po{h}", name=f"po{h}")
            for h in range(self.hpc)
        ]
        for j in range(njs):
            t = j - 4 * G
            ps = self.psum_s_pool.tile([128, 1024], self.f32, tag="ps", name="ps")
            q0 = 128 * t if (t > 0 and G >= 1) else 0
            for h in range(self.hpc):
                nc.tensor.matmul(
                    ps[:, h * 512 + q0 : (h + 1) * 512],
                    lhsT=self.kT[h * 64 : (h + 1) * 64, j * 128 : (j + 1) * 128],
                    rhs=self.qT[h * 64 : (h + 1) * 64, G * 512 + q0 : (G + 1) * 512],
                    start=True,
                    stop=True,
                    tile_position=(h * 64, 0),
                )
            pe = self.pexp_pool.tile([128, 1024], self.fp16, tag="pexp", name="pexp")
            if G == 0:
                # exact path with 0/1 mask multiplies (DVE, fp16 2x)
                self.exp_act_pair(pe, ps)
                for h in range(self.hpc):
                    nc.vector.tensor_mul(
                        pe[:, h * 512 : (h + 1) * 512],
                        pe[:, h * 512 : (h + 1) * 512],
                        self.mask01[t][:],
                    )
                    self.busy["dve"] += 512 * 0.521 + self.DVE_LAT
            elif t >= 0:
                self.exp_dve_pair(pe, ps, t, q0)
            elif self.busy["act"] <= self.busy["dve"]:
                self.exp_act_pair(pe, ps)
            else:
                self.exp_dve_pair(pe, ps, t, 0)
            self.pending.append((G, j, po, pe, njs, j == njs - 1))
            if len(self.pending) > 3:
                st = self.pending.pop(0)
                self.emit_mm2(st)
                if st[5]:
                    self.emit_finals(st[0], st[2])

    def flush(self):
        for st in self.pending:
            self.emit_mm2(st)
            if st[5]:
                self.emit_finals(st[0], st[2])
        self.pending = []


def _ensure_ntff_hook():
    """The image's antenv package lacks axon_hooks; provide it so
    run_bass_kernel_spmd's trace path works (or degrades gracefully)."""
    import sys
    import types

    try:
        import antenv.axon_hooks  # noqa: F401

        return
    except ImportError:
        pass
    mod = types.ModuleType("antenv.axon_hooks")
    state = {"hook": None}
    mod.set_axon_ntff_profile_hook = lambda h: state.__setitem__("hook", h)
    mod.get_axon_ntff_profile_hook = lambda: state["hook"]
    try:
        from trn_agent_boot.trn_boot import _ntff_profile_via_ctypes

        state["hook"] = _ntff_profile_via_ctypes("/opt/axon/libaxon_pjrt.so")
    except Exception:
        state["hook"] = None
    sys.modules["antenv.axon_hooks"] = mod


def kernel(q, k, v):
    """Full-input entry point: q, k, v [4096, 16, 64] fp32 -> [4096, 1024]."""
    import sys

    if "/opt/trn_rl_repo" not in sys.path:
        sys.path.insert(0, "/opt/trn_rl_repo")
    _ensure_ntff_hook()
    from concourse.bass_utils import run_bass_kernel_spmd

    q = np.asarray(q, dtype=np.float32)
    k = np.asarray(k, dtype=np.float32)
    v = np.asarray(v, dtype=np.float32)
    seq, nhead, hdim = q.shape

    if "nc" not in _NC_CACHE:
        _NC_CACHE["nc"] = build_attention_nc(seq=seq, hpc=HPC, hdim=hdim)
    nc = _NC_CACHE["nc"]

    in_maps = []
    for c in range(NCORES):
        hs = slice(c * HPC, (c + 1) * HPC)
        in_maps.append(
            {
                "q": np.ascontiguousarray(q[:, hs, :]),
                "k": np.ascontiguousarray(k[:, hs, :]),
                "v": np.ascontiguousarray(v[:, hs, :]),
            }
        )
    res = run_bass_kernel_spmd(nc, in_maps, core_ids=list(range(NCORES)))
    LAST_RESULT["exec_time_ns"] = res.exec_time_ns
    try:
        iat = res.instructions_and_trace
        LAST_RESULT["trace_path"] = iat[1] if iat else None
    except Exception:
        LAST_RESULT["trace_path"] = None
    outs = [res.results[c]["o"] for c in range(NCORES)]
    return np.concatenate(outs, axis=1)
